# revision 14
# baseline (speedup 1.0000x reference)
"""GNN message-passing kernel for 8 Trainium2 NeuronCores.

Full (unsharded) inputs in, full output out. Data-parallel over the batch
dimension: 64 graphs -> 8 cores x 8 graphs. Parameters replicated.

Math per graph (reference semantics):
  X  = W @ x^T                          [EMB=128, N=512]
  F1 = sigmoid(Wv @ X)                  [RP=64, N=512]
  Fn = F1
  repeat 7x:
    WwF = Ww @ Fn                       [64, 512]
    S   = Fn^T(r-contract) WwF          [512, 512]   S[n,m] = sum_r Fn[r,n] WwF[r,m]
    A   = softmax_n(mask ? -inf : S)
    Fn  = (Fn @ A) * F1
  gates q=0..7: g_q = sum_n sigmoid(Wg @ Fn_q)       [64]
  fT = concat(gates); fT /= ||fT||; out = MLP(fT)    [128]

Device implementation notes:
  * sigmoid(x) = 0.5 + 0.5*tanh(x/2)  -> tanh lives in the same ACT table set
    as exp, so no table switches in the main loop.
  * softmax without max-subtraction (S is bounded, exp(S-4) cannot overflow);
    multiplicative 0/1 adjacency mask applied to exp(S).  The column sums Z
    come out of the propagation matmul via an appended ones-column in lhsT.
  * gate sums come from ACT accum_out; they directly assemble the transposed
    feature matrix fTT [512, 8] used by the MLP.
  * rsqrt for the norm via bit-trick seed + Newton (no sqrt table needed).
  * matmuls run as float32r (TF32-like, full PE speed at N>=256).
"""

import os
import time

import numpy as np

import concourse.bacc as bacc
import concourse.tile as tile
import concourse.mybir as mybir
from concourse.bass_utils import run_bass_kernel_spmd

F32 = mybir.dt.float32
F32R = mybir.dt.float32r
if os.environ.get("GNN_MMDT") == "f32":
    F32R = mybir.dt.float32
U32 = mybir.dt.uint32
AF = mybir.ActivationFunctionType
ALU = mybir.AluOpType

B, N, FEAT, EMB, RP = 64, 512, 256, 128, 64
T = 8          # MAX_WALK_LEN (1 initial gate + 7 propagation steps)
NCORES = 8
BPC = B // NCORES   # graphs per core
NCH = N // 128      # n-chunks of 128
D0 = RP * T         # 512, MLP width

_STATE = {}


def ts(i, size):
    return slice(i * size, (i + 1) * size)


def _emit_phase_a(nc, p, g, gtiles, fnt_tiles):
    """X = W @ x^T, F1 = sigmoid(Wv X), gate 0, FnT(F1). Returns F1 tile."""
    xg = p["xg"].tile([128, 2 * N], F32R)
    for k in range(2):
        nc.sync.dma_start(xg[:, ts(k, N)], p["xT"][g, ts(k, 128), :])

    x_ps = p["s_ps"].tile([128, N], F32, tag="s", name="s_t")
    for k in range(2):
        nc.tensor.matmul(
            x_ps[:], p["wt_s"][:, ts(k, 128)],
            xg[:, ts(k, N)], start=(k == 0), stop=(k == 1))
    x_s = p["xs"].tile([128, N], F32R)
    nc.scalar.copy(x_s[:], x_ps[:])

    f1_ps = p["sm_ps"].tile([64, N], F32, tag="sm", name="sm_t")
    nc.tensor.matmul(f1_ps[:], p["wvt_s"][:], x_s[:], start=True, stop=True)
    scr = p["scr"].tile([64, N], F32)
    nc.scalar.activation(scr[:], f1_ps[:], AF.Tanh, scale=0.5)
    f1 = p["f1"].tile([64, N], F32R)
    nc.vector.tensor_scalar(f1[:], scr[:], 0.5, 0.5, ALU.mult, ALU.add)

    _emit_gate(nc, p, g, 0, f1, gtiles)
    _emit_fnt(nc, p, f1, fnt_tiles[0])
    if p["_dbg"] and g == 0:
        nc.sync.dma_start(p["d_f1"][:, :], f1[:].bitcast(F32))
        nc.sync.dma_start(p["d_fnt"][:, :], fnt_tiles[0][:].bitcast(F32))
    return f1


def _emit_gate(nc, p, g, q, fn, gtiles):
    """gate_q = sum_n sigmoid(Wg Fn) = 256 + 0.5*sum_n tanh(0.5*(Wg Fn)).
    Accumulates raw tanh-sum into the transposed-feature assembly tile;
    the affine (0.5, +256) is applied in the normalization phase."""
    gmm = p["prop_ps"].tile([65, N], F32, tag="pp", name="pp_t")
    nc.tensor.matmul(gmm[0:64, :], p["wgt_s"][:], fn[:], start=True, stop=True)
    scr = p["scr"].tile([64, N], F32)
    half = (q % 2) * 64
    acc = gtiles[q // 2][half:half + 64, g:g + 1]
    nc.scalar.activation(scr[:], gmm[0:64, :], AF.Tanh, scale=0.5,
                         accum_out=acc)


def _emit_fnt(nc, p, fn, fnt):
    """PE-transpose Fn [64,512] into fnt chunks [128,64] (cols j*65..j*65+63).
    Column j*65+64 holds the persistent ones used for the Z row."""
    ident = p["identr_s"]
    fnt_ps = p["fnt_ps"].tile([128, 4 * 64], F32R, tag="ft", name="ft_t")
    for j in range(NCH):
        nc.tensor.transpose(fnt_ps[:, ts(j, 64)], fn[:, ts(j, 128)],
                            ident[0:64, 0:64])
    for j in range(NCH):
        nc.vector.tensor_copy(fnt[:, j * 65:j * 65 + 64], fnt_ps[:, ts(j, 64)])


def _emit_iter(nc, p, g, t, fn_prev, fnt_prev, fnt_next, f1, adj_g, gtiles,
               mask_engines):
    """One propagation step for graph g. Returns the new Fn tile."""
    # WwF = WwT.T @ Fn  -> [64, 512], copy to SBUF (matmul lhsT must be SBUF)
    wwf_ps = p["sm_ps"].tile([64, N], F32, tag="sm", name="sm_t")
    nc.tensor.matmul(wwf_ps[:], p["wwt_s"][:], fn_prev[:], start=True, stop=True)
    wwf_s = p["wwf"].tile([64, N], F32R)
    nc.scalar.copy(wwf_s[:], wwf_ps[:])

    # S chunks -> exp -> mask; then propagation accumulate with Z row
    prop = p["prop_ps"].tile([65, N], F32, tag="pp", name="pp_t")
    for j in range(NCH):
        s_ps = p["s_ps"].tile([128, N], F32, tag="s", name="s_t")
        nc.tensor.matmul(s_ps[:], fn_prev[:, ts(j, 128)], wwf_s[:], start=True, stop=True)
        e_t = p["e"].tile([128, N], F32)
        nc.scalar.activation(e_t[:], s_ps[:], AF.Exp)
        em_t = p["em"].tile([128, N], F32R)
        eng = nc.vector if mask_engines[j] == "v" else nc.gpsimd
        eng.tensor_tensor(em_t[:], e_t[:], adj_g[:, ts(j, N)], ALU.mult)
        if p["_dbg"] and g == 0 and t == 1 and j == 0:
            nc.sync.dma_start(p["d_em0"][:, :], em_t[:].bitcast(F32))
        nc.tensor.matmul(prop[:], fnt_prev[:, j * 65:(j + 1) * 65],
                         em_t[:], start=(j == 0), stop=(j == 3))

    # normalization: rz = 1/Z broadcast onto 64 partitions via K=1 matmul
    zsb = p["zsb"].tile([1, N], F32)
    nc.scalar.copy(zsb[:], prop[64:65, :])
    rz = p["rz"].tile([1, N], F32)
    nc.vector.reciprocal_approx_fast(rz[:], zsb[:])
    if p["_dbg"] and g == 0 and t == 1:
        nc.sync.dma_start(p["d_rz"][:, :], rz[:])
        dbg_s = p["scr"].tile([65, N], F32, tag="dbgs", name="dbgs")
        nc.vector.tensor_copy(dbg_s[:], prop[:])
        nc.sync.dma_start(p["d_prop"][:, :], dbg_s[:])
        nc.sync.dma_start(p["d_wwf"][:, :], wwf_s[:].bitcast(F32))
    rz_r = p["rzr"].tile([1, N], F32R)
    nc.vector.tensor_copy(rz_r[:], rz[:])
    fnuf1 = p["fnuf1"].tile([64, N], F32)
    nc.vector.tensor_tensor(fnuf1[:], prop[0:64, :], f1[:], ALU.mult)
    zb_ps = p["sm_ps"].tile([64, N], F32, tag="sm", name="sm_t")
    nc.tensor.matmul(zb_ps[:], p["ones_s"][0:1, 0:64], rz_r[:], start=True, stop=True)
    fn_new = p["fn"].tile([64, N], F32R)
    nc.vector.tensor_tensor(fn_new[:], zb_ps[:], fnuf1[:], ALU.mult)

    if p["_dbg"] and g == 0 and t == 1:
        nc.sync.dma_start(p["d_fn1"][:, :], fn_new[:].bitcast(F32))
    _emit_gate(nc, p, g, t, fn_new, gtiles)
    if t < T - 1:
        _emit_fnt(nc, p, fn_new, fnt_next)
    return fn_new


def _emit_norm_mlp(nc, p, gtiles):
    """Gate affine + L2 normalization + 4-layer MLP, all graphs at once."""
    ident = p["ident_s"]
    # transpose the 4 assembly tiles [128, BPC] -> row layout [BPC, 512]
    row_ps = p["sm_ps"].tile([BPC, N], F32, tag="sm", name="sm_t")
    for j in range(NCH):
        nc.tensor.transpose(row_ps[:, ts(j, 128)], gtiles[j][:, 0:BPC],
                            ident[:, 0:128])
    f_row = p["frow"].tile([BPC, N], F32)
    # gate = 0.5*acc + 256 applied during the PSUM->SBUF copy
    nc.vector.tensor_scalar(f_row[:], row_ps[:], 0.5, 256.0, ALU.mult, ALU.add)
    if p["_dbg"]:
        nc.sync.dma_start(p["d_frow"][:, :], f_row[:])
        for j in range(4):
            nc.sync.dma_start(p["d_gt"][:, ts(j, BPC)], gtiles[j][:, 0:BPC])

    # ss[g] = sum_k f_row[g,k]^2 via Square activation with accumulate
    sq = p["frow"].tile([BPC, N], F32)
    ss = p["tiny"].tile([BPC, 1], F32, tag="ss")
    nc.scalar.activation(sq[:], f_row[:], AF.Square, accum_out=ss[:])

    # rn = rsqrt(ss): bit-trick sqrt seed, fast reciprocal, 3 Newton steps
    tmp = p["tiny"].tile([BPC, 1], F32, tag="t0")
    y = p["tiny"].tile([BPC, 1], F32, tag="t1")
    a = p["tiny"].tile([BPC, 1], F32, tag="t2")
    nc.vector.tensor_scalar(tmp[:].bitcast(U32), ss[:].bitcast(U32),
                            1, None, ALU.logical_shift_right)
    nc.vector.tensor_scalar(tmp[:].bitcast(U32), tmp[:].bitcast(U32),
                            0x1FBD1DF5, None, ALU.add)
    nc.vector.reciprocal_approx_fast(y[:], tmp[:])
    for _ in range(3):
        nc.vector.tensor_tensor(a[:], y[:], y[:], ALU.mult)       # y^2
        nc.vector.tensor_tensor(a[:], ss[:], a[:], ALU.mult)      # ss*y^2
        nc.vector.tensor_scalar(a[:], a[:], -0.5, 1.5, ALU.mult, ALU.add)
        nc.vector.tensor_tensor(y[:], y[:], a[:], ALU.mult)

    fn_row = p["frow"].tile([BPC, N], F32)
    nc.vector.tensor_scalar(fn_row[:], f_row[:], y[:], None, ALU.mult)
    if p["_dbg"]:
        nc.sync.dma_start(p["d_fnrow"][:, :], fn_row[:])

    # back to transposed layout [512, BPC] for the MLP
    h0 = p["mlp"].tile([128, 4 * BPC], F32, tag="h0")
    for j in range(NCH):
        t_ps = p["fnt_ps"].tile([128, 4 * 64], F32, tag="ft", name="ft_t")
        nc.tensor.transpose(t_ps[:, 0:BPC], fn_row[:, ts(j, 128)],
                            ident[0:BPC, 0:BPC])
        nc.vector.tensor_copy(h0[:, ts(j, BPC)], t_ps[:, 0:BPC])

    # MLP layers in transposed layout: h_next[j,g] = act(sum_k WT[k,j] h[k,g] + b[j])
    def layer(h_in, kch, jch, w_s, b_s, act, tag):
        h_out = p["mlp"].tile([128, jch * BPC], F32, tag=tag)
        for j in range(jch):
            mm = p["sm_ps"].tile([128, BPC], F32, tag="sm", name="sm_t")
            for k in range(kch):
                nc.tensor.matmul(mm[:], w_s[:, k * (jch * 128) + j * 128:
                                             k * (jch * 128) + (j + 1) * 128],
                                 h_in[:, ts(k, BPC)],
                                 start=(k == 0), stop=(k == kch - 1))
            nc.scalar.activation(h_out[:, ts(j, BPC)], mm[:], act,
                                 bias=b_s[:, j:j + 1])
        return h_out

    h1 = layer(h0, 4, 4, p["w0t_s"], p["b0_s"], AF.Relu, "h1")
    if p["_dbg"]:
        nc.sync.dma_start(p["d_h1"][:, :], h1[:])
    h2 = layer(h1, 4, 4, p["w1t_s"], p["b1_s"], AF.Relu, "h2")
    h3 = layer(h2, 4, 2, p["w2t_s"], p["b2_s"], AF.Relu, "h3")
    h4 = layer(h3, 2, 1, p["w3t_s"], p["b3_s"], AF.Identity, "h4")
    nc.sync.dma_start(p["outT"][:, :], h4[:, 0:BPC])


def _build_program(mask_engines="vgvg"):
    nc = bacc.Bacc("TRN2", target_bir_lowering=False, debug=False,
                   num_devices=NCORES)
    p = {}
    p["xT"] = nc.dram_tensor("xT", [BPC, FEAT, N], F32R, kind="ExternalInput").ap()
    p["adjf"] = nc.dram_tensor("adjf", [BPC, N, N], F32, kind="ExternalInput").ap()
    for name, shape, dt in [("wt", [FEAT, EMB], F32R), ("wvt", [EMB, RP], F32R),
                            ("wwt", [RP, RP], F32R), ("wgt", [RP, RP], F32R),
                            ("w0t", [D0, D0], F32), ("w1t", [D0, D0], F32),
                            ("w2t", [D0, D0 // 2], F32),
                            ("w3t", [D0 // 2, 128], F32),
                            ("b0", [D0], F32), ("b1", [D0], F32),
                            ("b2", [D0 // 2], F32), ("b3", [128], F32),
                            ("ident", [128, 128], F32),
                            ("identr", [128, 128], F32R)]:
        p[name] = nc.dram_tensor(name, shape, dt, kind="ExternalInput").ap()
    p["outT"] = nc.dram_tensor("outT", [128, BPC], F32, kind="ExternalOutput").ap()
    p["_dbg"] = bool(int(os.environ.get("GNN_DEBUG", "0")))
    if p["_dbg"]:
        for nm, shp in [("d_f1", [64, N]), ("d_fn1", [64, N]), ("d_wwf", [64, N]),
                        ("d_em0", [128, N]), ("d_rz", [1, N]), ("d_prop", [65, N]),
                        ("d_frow", [BPC, N]), ("d_fnrow", [BPC, N]),
                        ("d_gt", [128, 4 * BPC]), ("d_h1", [128, 4 * BPC]),
                        ("d_fnt", [128, 4 * 65])]:
            p[nm] = nc.dram_tensor(nm, shp, F32, kind="ExternalOutput").ap()

    with tile.TileContext(nc) as tc:
        import contextlib
        with contextlib.ExitStack() as ctx:
            # persistent pools
            const = ctx.enter_context(tc.tile_pool(name="const", bufs=1))
            p["wt_s"] = const.tile([128, 2 * 128], F32R, tag="wt", name="wt_s")
            p["wvt_s"] = const.tile([128, 64], F32R, tag="wvt", name="wvt_s")
            p["wwt_s"] = const.tile([64, 64], F32R, tag="wwt", name="wwt_s")
            p["wgt_s"] = const.tile([64, 64], F32R, tag="wgt", name="wgt_s")
            p["ident_s"] = const.tile([128, 128], F32, tag="ident", name="ident_s")
            p["identr_s"] = const.tile([128, 128], F32R, tag="identr", name="identr_s")
            p["ones_s"] = const.tile([1, 64], F32R, tag="ones", name="ones_s")
            p["w0t_s"] = const.tile([128, 4 * D0], F32, tag="w0t", name="w0t_s")
            p["w1t_s"] = const.tile([128, 4 * D0], F32, tag="w1t", name="w1t_s")
            p["w2t_s"] = const.tile([128, 4 * (D0 // 2)], F32, tag="w2t", name="w2t_s")
            p["w3t_s"] = const.tile([128, 2 * 128], F32, tag="w3t", name="w3t_s")
            p["b0_s"] = const.tile([128, 4], F32, tag="b0", name="b0_s")
            p["b1_s"] = const.tile([128, 4], F32, tag="b1", name="b1_s")
            p["b2_s"] = const.tile([128, 2], F32, tag="b2", name="b2_s")
            p["b3_s"] = const.tile([128, 1], F32, tag="b3", name="b3_s")
            gtiles = [const.tile([128, BPC], F32, tag=f"gt{i}", name=f"gt{i}") for i in range(4)]
            fnt_tiles = [const.tile([128, 4 * 65], F32R, tag=f"fnt{i}", name=f"fnt{i}")
                         for i in range(4)]  # 2 ping-pong per graph of a pair

            # weight loads
            for k in range(2):
                nc.sync.dma_start(p["wt_s"][:, ts(k, 128)], p["wt"][ts(k, 128), :])
            nc.sync.dma_start(p["wvt_s"][:], p["wvt"][:, :])
            nc.sync.dma_start(p["wwt_s"][:], p["wwt"][:, :])
            nc.sync.dma_start(p["wgt_s"][:], p["wgt"][:, :])
            nc.sync.dma_start(p["ident_s"][:], p["ident"][:, :])
            nc.sync.dma_start(p["identr_s"][:], p["identr"][:, :])
            nc.gpsimd.memset(p["ones_s"][:].bitcast(U32), 0x3F800000)
            for k in range(4):
                nc.sync.dma_start(p["w0t_s"][:, ts(k, D0)], p["w0t"][ts(k, 128), :])
                nc.sync.dma_start(p["w1t_s"][:, ts(k, D0)], p["w1t"][ts(k, 128), :])
                nc.sync.dma_start(p["w2t_s"][:, ts(k, D0 // 2)],
                                  p["w2t"][ts(k, 128), :])
                nc.sync.dma_start(p["b0_s"][:, k:k + 1], p["b0"][ts(k, 128)])
                nc.sync.dma_start(p["b1_s"][:, k:k + 1], p["b1"][ts(k, 128)])
            for k in range(2):
                nc.sync.dma_start(p["w3t_s"][:, ts(k, 128)], p["w3t"][ts(k, 128), :])
                nc.sync.dma_start(p["b2_s"][:, k:k + 1], p["b2"][ts(k, 128)])
            nc.sync.dma_start(p["b3_s"][:, 0:1], p["b3"][:])
            for i in range(4):
                nc.gpsimd.memset(gtiles[i][:], 0.0)
                for j in range(NCH):
                    nc.gpsimd.memset(
                        fnt_tiles[i][:, j * 65 + 64:j * 65 + 65].bitcast(U32),
                        0x3F800000)

            # working pools
            p["xg"] = ctx.enter_context(tc.tile_pool(name="xg", bufs=2))
            p["adj"] = ctx.enter_context(tc.tile_pool(name="adj", bufs=3))
            p["xs"] = ctx.enter_context(tc.tile_pool(name="xs", bufs=2))
            p["f1"] = ctx.enter_context(tc.tile_pool(name="f1", bufs=2))
            p["fn"] = ctx.enter_context(tc.tile_pool(name="fn", bufs=4))
            p["scr"] = ctx.enter_context(tc.tile_pool(name="scr", bufs=3))
            p["wwf"] = ctx.enter_context(tc.tile_pool(name="wwf", bufs=3))
            p["e"] = ctx.enter_context(tc.tile_pool(name="e", bufs=6))
            p["em"] = ctx.enter_context(tc.tile_pool(name="em", bufs=6))
            p["rz"] = ctx.enter_context(tc.tile_pool(name="rz", bufs=2))
            p["zsb"] = ctx.enter_context(tc.tile_pool(name="zsb", bufs=2))
            p["rzr"] = ctx.enter_context(tc.tile_pool(name="rzr", bufs=2))
            p["fnuf1"] = ctx.enter_context(tc.tile_pool(name="fnuf1", bufs=2))
            p["frow"] = ctx.enter_context(tc.tile_pool(name="frow", bufs=1))
            p["tiny"] = ctx.enter_context(tc.tile_pool(name="tiny", bufs=1))
            p["mlp"] = ctx.enter_context(tc.tile_pool(name="mlp", bufs=1))
            # PSUM: 2 + 2 + 2 + 2 = 8 banks
            p["s_ps"] = ctx.enter_context(
                tc.tile_pool(name="s_ps", bufs=2, space="PSUM"))
            p["prop_ps"] = ctx.enter_context(
                tc.tile_pool(name="prop_ps", bufs=3, space="PSUM"))
            p["sm_ps"] = ctx.enter_context(
                tc.tile_pool(name="sm_ps", bufs=2, space="PSUM"))
            p["fnt_ps"] = ctx.enter_context(
                tc.tile_pool(name="fnt_ps", bufs=1, space="PSUM"))

            # main loop: graphs in interleaved pairs
            for gp in range(BPC // 2):
                pair = (2 * gp, 2 * gp + 1)
                adj_g, f1_g, fn_g = {}, {}, {}
                for i, g in enumerate(pair):
                    ag = p["adj"].tile([128, NCH * N], F32)
                    for j in range(NCH):
                        nc.sync.dma_start(ag[:, ts(j, N)], p["adjf"][g, ts(j, 128), :])
                    adj_g[g] = ag
                for i, g in enumerate(pair):
                    f1_g[g] = _emit_phase_a(nc, p, g, gtiles,
                                            fnt_tiles[2 * i:2 * i + 2])
                    fn_g[g] = f1_g[g]
                for t in range(1, T):
                    for i, g in enumerate(pair):
                        fnt_pair = fnt_tiles[2 * i:2 * i + 2]
                        fn_g[g] = _emit_iter(
                            nc, p, g, t, fn_g[g],
                            fnt_pair[(t - 1) % 2], fnt_pair[t % 2],
                            f1_g[g], adj_g[g], gtiles, "vgvg")

            _emit_norm_mlp(nc, p, gtiles)

    nc.compile()
    return nc


def _prep_inputs(inputs):
    x = np.ascontiguousarray(np.asarray(inputs["node_attribute_matrix"], np.float32))
    adj = np.asarray(inputs["adjacent_matrix"])
    adjf = np.ascontiguousarray((adj != 0).astype(np.float32))
    xT = np.ascontiguousarray(x.transpose(0, 2, 1))  # [B, FEAT, N]

    common = {
        "wt": np.ascontiguousarray(np.asarray(inputs["W"], np.float32).T),
        "wvt": np.ascontiguousarray(np.asarray(inputs["Wv"], np.float32).T),
        "wwt": np.ascontiguousarray(np.asarray(inputs["Ww"], np.float32).T),
        "wgt": np.ascontiguousarray(np.asarray(inputs["Wg"], np.float32).T),
        "w0t": np.ascontiguousarray(np.asarray(inputs["W0"], np.float32).T),
        "w1t": np.ascontiguousarray(np.asarray(inputs["W1"], np.float32).T),
        "w2t": np.ascontiguousarray(np.asarray(inputs["W2"], np.float32).T),
        "w3t": np.ascontiguousarray(np.asarray(inputs["W3"], np.float32).T),
        "b0": np.asarray(inputs["b0"], np.float32),
        "b1": np.asarray(inputs["b1"], np.float32),
        "b2": np.asarray(inputs["b2"], np.float32),
        "b3": np.asarray(inputs["b3"], np.float32),
        "ident": np.eye(128, dtype=np.float32),
        "identr": np.eye(128, dtype=np.float32),
    }
    in_maps = []
    for c in range(NCORES):
        sl = slice(c * BPC, (c + 1) * BPC)
        m = dict(common)
        m["xT"] = xT[sl]
        m["adjf"] = adjf[sl]
        in_maps.append(m)
    return in_maps


def kernel(**inputs) -> np.ndarray:
    if "nc" not in _STATE:
        _STATE["nc"] = _build_program()
    nc = _STATE["nc"]
    in_maps = _prep_inputs(inputs)

    trace = bool(int(os.environ.get("GNN_TRACE", "0")))
    kwargs = {}
    if trace:
        kwargs = dict(trace=True, tmpdir=os.environ.get("GNN_TRACE_DIR") or None)
    t0 = time.time()
    res = run_bass_kernel_spmd(nc, in_maps, list(range(NCORES)), **kwargs)
    _STATE["wall_s"] = time.time() - t0
    _STATE["exec_time_ns"] = res.exec_time_ns
    _STATE["results"] = res

    out = np.empty((B, 128), np.float32)
    for c in range(NCORES):
        out[c * BPC:(c + 1) * BPC] = res.results[c]["outT"].T
    return out


# revision 16
# speedup vs baseline: 1.0619x; 1.0619x over previous
"""GNN message-passing kernel for 8 Trainium2 NeuronCores.

Full (unsharded) inputs in, full output out. Data-parallel over the batch
dimension: 64 graphs -> 8 cores x 8 graphs. Parameters replicated.

Math per graph (reference semantics):
  X  = W @ x^T                          [EMB=128, N=512]
  F1 = sigmoid(Wv @ X)                  [RP=64, N=512]
  Fn = F1
  repeat 7x:
    WwF = Ww @ Fn                       [64, 512]
    S   = Fn^T(r-contract) WwF          [512, 512]   S[n,m] = sum_r Fn[r,n] WwF[r,m]
    A   = softmax_n(mask ? -inf : S)
    Fn  = (Fn @ A) * F1
  gates q=0..7: g_q = sum_n sigmoid(Wg @ Fn_q)       [64]
  fT = concat(gates); fT /= ||fT||; out = MLP(fT)    [128]

Device implementation notes:
  * loop tensors are bf16 (validated: output error stays ~1e-4); PSUM
    accumulation is fp32 throughout.
  * sigmoid(x) = 0.5 + 0.5*tanh(x/2)  -> tanh lives in the same ACT table set
    as exp, so no table switches in the main loop.
  * softmax without max-subtraction (S is bounded); multiplicative 0/1
    adjacency mask applied to exp(S) in one fused [128, 2048] DVE op (2x
    bf16 mode).  The four S chunks live in one 4-bank PSUM tile so exp is a
    single [128, 2048] ACT op.
  * column sums Z come from an appended ones-column in the propagation
    matmul lhsT; 1/Z via fast DVE reciprocal (input staged to SBUF - the
    custom op misreads PSUM re-reads); broadcast via gpsimd.
  * gate sums come from ACT accum_out, directly assembling the transposed
    feature matrix fTT [512, 8] used by the MLP.
  * rsqrt for the norm via bit-trick seed + Newton (no sqrt table needed).
"""

import os
import time

import ml_dtypes
import numpy as np

import concourse.bacc as bacc
import concourse.tile as tile
import concourse.mybir as mybir
from concourse.bass_utils import run_bass_kernel_spmd

F32 = mybir.dt.float32
BF16 = mybir.dt.bfloat16
U32 = mybir.dt.uint32
AF = mybir.ActivationFunctionType
ALU = mybir.AluOpType

B, N, FEAT, EMB, RP = 64, 512, 256, 128, 64
T = 8          # MAX_WALK_LEN (1 initial gate + 7 propagation steps)
NCORES = 8
BPC = B // NCORES   # graphs per core
NCH = N // 128      # n-chunks of 128
D0 = RP * T         # 512, MLP width

_STATE = {}


def ts(i, size):
    return slice(i * size, (i + 1) * size)


def _emit_phase_a(nc, p, g, gtiles, fnt_tiles):
    """X = W @ x^T, F1 = sigmoid(Wv X), gate 0, FnT(F1). Returns F1 tile."""
    xg = p["xg"].tile([128, 2 * N], BF16)
    for k in range(2):
        nc.sync.dma_start(xg[:, ts(k, N)], p["xT"][g, ts(k, 128), :])

    x_ps = p["sm_ps"].tile([128, N], F32, tag="sm", name="sm_t")
    for k in range(2):
        nc.tensor.matmul(x_ps[:], p["wt_s"][:, ts(k, 128)], xg[:, ts(k, N)],
                         start=(k == 0), stop=(k == 1))
    x_s = p["xs"].tile([128, N], BF16)
    nc.scalar.copy(x_s[:], x_ps[:])

    f1_ps = p["sm_ps"].tile([64, N], F32, tag="sm", name="sm_t")
    nc.tensor.matmul(f1_ps[:], p["wvt_s"][:], x_s[:], start=True, stop=True)
    scr = p["scr"].tile([64, N], F32)
    nc.scalar.activation(scr[:], f1_ps[:], AF.Tanh, scale=0.5)
    f1 = p["f1"].tile([64, N], BF16)
    nc.vector.tensor_scalar(f1[:], scr[:], 0.5, 0.5, ALU.mult, ALU.add)

    _emit_gate(nc, p, g, 0, f1, gtiles)
    _emit_fnt(nc, p, f1, fnt_tiles[0])
    return f1


def _emit_gate(nc, p, g, q, fn, gtiles):
    """gate_q = sum_n sigmoid(Wg Fn) = 256 + 0.5*sum_n tanh(0.5*(Wg Fn)).
    Raw tanh-sum accumulates into the transposed-feature assembly tile;
    the affine (0.5, +256) is applied in the normalization phase."""
    gmm = p["prop_ps"].tile([65, N], F32, tag="pp", name="pp_t")
    nc.tensor.matmul(gmm[0:64, :], p["wgt_s"][:], fn[:], start=True, stop=True)
    scr = p["scr"].tile([64, N], F32)
    half = (q % 2) * 64
    acc = gtiles[q // 2][half:half + 64, g:g + 1]
    nc.scalar.activation(scr[:], gmm[0:64, :], AF.Tanh, scale=0.5,
                         accum_out=acc)


def _emit_fnt(nc, p, fn, fnt):
    """PE-transpose Fn [64,512] into fnt chunks [128,64] (cols j*65..j*65+63).
    Column j*65+64 holds the persistent ones used for the Z row."""
    fnt_ps = p["fnt_ps"].tile([128, 4 * 64], BF16, tag="sm", name="sm_t")
    for j in range(NCH):
        nc.tensor.transpose(fnt_ps[:, ts(j, 64)], fn[:, ts(j, 128)],
                            p["identb_s"][0:64, 0:64])
    # one strided copy: chunks land at stride-65 offsets, skipping ones cols
    dst = fnt.rearrange("p (c k) -> p c k", k=65)[:, :, 0:64]
    src = fnt_ps[:].rearrange("p (c k) -> p c k", k=64)
    nc.vector.tensor_copy(dst, src)


def _emit_iter(nc, p, g, t, fn_prev, fnt_prev, fnt_next, f1, adj_g, gtiles):
    """One propagation step for graph g. Returns the new Fn tile."""
    # WwF = WwT.T @ Fn  -> [64, 512], copy to SBUF (matmul lhsT must be SBUF)
    wwf_ps = p["sm_ps"].tile([64, N], F32, tag="sm", name="sm_t")
    nc.tensor.matmul(wwf_ps[:], p["wwt_s"][:], fn_prev[:], start=True, stop=True)
    wwf_s = p["wwf"].tile([64, N], BF16)
    nc.scalar.copy(wwf_s[:], wwf_ps[:])

    # four S chunks into one 4-bank PSUM tile, then fused exp and mask
    s_all = p["s_ps"].tile([128, NCH * N], F32, tag="s", name="s_t")
    for j in range(NCH):
        nc.tensor.matmul(s_all[:, ts(j, N)], fn_prev[:, ts(j, 128)], wwf_s[:],
                         start=True, stop=True)
    e_all = p["e"].tile([128, NCH * N], BF16)
    nc.scalar.activation(e_all[:], s_all[:], AF.Exp)
    em_all = p["em"].tile([128, NCH * N], BF16)
    nc.vector.tensor_tensor(em_all[:], e_all[:], adj_g[:], ALU.mult)

    prop = p["prop_ps"].tile([65, N], F32, tag="pp", name="pp_t")
    for j in range(NCH):
        nc.tensor.matmul(prop[:], fnt_prev[:, j * 65:(j + 1) * 65],
                         em_all[:, ts(j, N)], start=(j == 0), stop=(j == 3))

    # softmax normalization: rz = 1/Z, broadcast across partitions via gpsimd
    zsb = p["zsb"].tile([1, N], F32)
    nc.scalar.copy(zsb[:], prop[64:65, :])
    rz = p["rz"].tile([1, N], F32)
    nc.vector.reciprocal_approx_fast(rz[:], zsb[:])
    zb = p["zb"].tile([64, N], F32)
    nc.gpsimd.partition_broadcast(zb[:], rz[:])
    fnuf1 = p["fnuf1"].tile([64, N], F32)
    nc.vector.tensor_tensor(fnuf1[:], prop[0:64, :], f1[:], ALU.mult)
    fn_new = p["fn"].tile([64, N], BF16)
    nc.vector.tensor_tensor(fn_new[:], fnuf1[:], zb[:], ALU.mult)

    _emit_gate(nc, p, g, t, fn_new, gtiles)
    if t < T - 1:
        _emit_fnt(nc, p, fn_new, fnt_next)
    return fn_new


def _emit_norm_mlp(nc, p, gtiles):
    """Gate affine + L2 normalization + 4-layer MLP, all graphs at once."""
    ident = p["ident_s"]
    # transpose the 4 assembly tiles [128, BPC] -> row layout [BPC, 512]
    row_ps = p["sm_ps"].tile([BPC, N], F32, tag="sm", name="sm_t")
    for j in range(NCH):
        nc.tensor.transpose(row_ps[:, ts(j, 128)], gtiles[j][:, 0:BPC],
                            ident[:, 0:128])
    f_row = p["frow"].tile([BPC, N], F32)
    # gate = 0.5*acc + 256 applied during the PSUM->SBUF copy
    nc.vector.tensor_scalar(f_row[:], row_ps[:], 0.5, 256.0, ALU.mult, ALU.add)

    # ss[g] = sum_k f_row[g,k]^2 via Square activation with accumulate
    sq = p["frow"].tile([BPC, N], F32)
    ss = p["tiny"].tile([BPC, 1], F32, tag="ss")
    nc.scalar.activation(sq[:], f_row[:], AF.Square, accum_out=ss[:])

    # rn = rsqrt(ss): bit-trick sqrt seed, fast reciprocal, 3 Newton steps
    tmp = p["tiny"].tile([BPC, 1], F32, tag="t0")
    y = p["tiny"].tile([BPC, 1], F32, tag="t1")
    a = p["tiny"].tile([BPC, 1], F32, tag="t2")
    nc.vector.tensor_scalar(tmp[:].bitcast(U32), ss[:].bitcast(U32),
                            1, None, ALU.logical_shift_right)
    nc.vector.tensor_scalar(tmp[:].bitcast(U32), tmp[:].bitcast(U32),
                            0x1FBD1DF5, None, ALU.add)
    nc.vector.reciprocal_approx_fast(y[:], tmp[:])
    for _ in range(3):
        nc.vector.tensor_tensor(a[:], y[:], y[:], ALU.mult)       # y^2
        nc.vector.tensor_tensor(a[:], ss[:], a[:], ALU.mult)      # ss*y^2
        nc.vector.tensor_scalar(a[:], a[:], -0.5, 1.5, ALU.mult, ALU.add)
        nc.vector.tensor_tensor(y[:], y[:], a[:], ALU.mult)

    fn_row = p["frow"].tile([BPC, N], F32)
    nc.vector.tensor_scalar(fn_row[:], f_row[:], y[:], None, ALU.mult)

    # back to transposed layout [512, BPC] for the MLP
    h0 = p["mlp"].tile([128, 4 * BPC], F32, tag="h0")
    for j in range(NCH):
        t_ps = p["sm_ps"].tile([128, BPC], F32, tag="sm", name="sm_t")
        nc.tensor.transpose(t_ps[:, 0:BPC], fn_row[:, ts(j, 128)],
                            ident[0:BPC, 0:BPC])
        nc.vector.tensor_copy(h0[:, ts(j, BPC)], t_ps[:, 0:BPC])

    # MLP in transposed layout: h_next[j,g] = act(sum_k WT[k,j] h[k,g] + b[j])
    def layer(h_in, kch, jch, w_s, b_s, act, tag):
        h_out = p["mlp"].tile([128, jch * BPC], F32, tag=tag)
        for j in range(jch):
            mm = p["sm_ps"].tile([128, BPC], F32, tag="sm", name="sm_t")
            for k in range(kch):
                nc.tensor.matmul(mm[:], w_s[:, k * (jch * 128) + j * 128:
                                             k * (jch * 128) + (j + 1) * 128],
                                 h_in[:, ts(k, BPC)],
                                 start=(k == 0), stop=(k == kch - 1))
            nc.scalar.activation(h_out[:, ts(j, BPC)], mm[:], act,
                                 bias=b_s[:, j:j + 1])
        return h_out

    h1 = layer(h0, 4, 4, p["w0t_s"], p["b0_s"], AF.Relu, "h1")
    h2 = layer(h1, 4, 4, p["w1t_s"], p["b1_s"], AF.Relu, "h2")
    h3 = layer(h2, 4, 2, p["w2t_s"], p["b2_s"], AF.Relu, "h3")
    h4 = layer(h3, 2, 1, p["w3t_s"], p["b3_s"], AF.Identity, "h4")
    nc.sync.dma_start(p["outT"][:, :], h4[:, 0:BPC])


def _build_program():
    nc = bacc.Bacc("TRN2", target_bir_lowering=False, debug=False,
                   num_devices=NCORES)
    p = {}
    p["xT"] = nc.dram_tensor("xT", [BPC, FEAT, N], BF16, kind="ExternalInput").ap()
    p["adjf"] = nc.dram_tensor("adjf", [BPC, N, N], BF16, kind="ExternalInput").ap()
    for name, shape, dt in [("wt", [FEAT, EMB], BF16), ("wvt", [EMB, RP], BF16),
                            ("wwt", [RP, RP], BF16), ("wgt", [RP, RP], BF16),
                            ("w0t", [D0, D0], F32), ("w1t", [D0, D0], F32),
                            ("w2t", [D0, D0 // 2], F32),
                            ("w3t", [D0 // 2, 128], F32),
                            ("b0", [D0], F32), ("b1", [D0], F32),
                            ("b2", [D0 // 2], F32), ("b3", [128], F32),
                            ("ident", [128, 128], F32),
                            ("identb", [128, 128], BF16)]:
        p[name] = nc.dram_tensor(name, shape, dt, kind="ExternalInput").ap()
    p["outT"] = nc.dram_tensor("outT", [128, BPC], F32, kind="ExternalOutput").ap()

    with tile.TileContext(nc) as tc:
        import contextlib
        with contextlib.ExitStack() as ctx:
            const = ctx.enter_context(tc.tile_pool(name="const", bufs=1))
            p["wt_s"] = const.tile([128, 2 * 128], BF16, tag="wt", name="wt_s")
            p["wvt_s"] = const.tile([128, 64], BF16, tag="wvt", name="wvt_s")
            p["wwt_s"] = const.tile([64, 64], BF16, tag="wwt", name="wwt_s")
            p["wgt_s"] = const.tile([64, 64], BF16, tag="wgt", name="wgt_s")
            p["ident_s"] = const.tile([128, 128], F32, tag="ident", name="ident_s")
            p["identb_s"] = const.tile([128, 128], BF16, tag="identb",
                                       name="identb_s")
            p["w0t_s"] = const.tile([128, 4 * D0], F32, tag="w0t", name="w0t_s")
            p["w1t_s"] = const.tile([128, 4 * D0], F32, tag="w1t", name="w1t_s")
            p["w2t_s"] = const.tile([128, 4 * (D0 // 2)], F32, tag="w2t",
                                    name="w2t_s")
            p["w3t_s"] = const.tile([128, 2 * 128], F32, tag="w3t", name="w3t_s")
            p["b0_s"] = const.tile([128, 4], F32, tag="b0", name="b0_s")
            p["b1_s"] = const.tile([128, 4], F32, tag="b1", name="b1_s")
            p["b2_s"] = const.tile([128, 2], F32, tag="b2", name="b2_s")
            p["b3_s"] = const.tile([128, 1], F32, tag="b3", name="b3_s")
            gtiles = [const.tile([128, BPC], F32, tag=f"gt{i}", name=f"gt{i}")
                      for i in range(4)]
            fnt_tiles = [const.tile([128, 4 * 65], BF16, tag=f"fnt{i}",
                                    name=f"fnt{i}")
                         for i in range(4)]  # 2 ping-pong per graph of a pair

            for k in range(2):
                nc.sync.dma_start(p["wt_s"][:, ts(k, 128)], p["wt"][ts(k, 128), :])
            nc.sync.dma_start(p["wvt_s"][:], p["wvt"][:, :])
            nc.sync.dma_start(p["wwt_s"][:], p["wwt"][:, :])
            nc.sync.dma_start(p["wgt_s"][:], p["wgt"][:, :])
            nc.sync.dma_start(p["ident_s"][:], p["ident"][:, :])
            nc.sync.dma_start(p["identb_s"][:], p["identb"][:, :])
            for k in range(4):
                nc.sync.dma_start(p["w0t_s"][:, ts(k, D0)], p["w0t"][ts(k, 128), :])
                nc.sync.dma_start(p["w1t_s"][:, ts(k, D0)], p["w1t"][ts(k, 128), :])
                nc.sync.dma_start(p["w2t_s"][:, ts(k, D0 // 2)],
                                  p["w2t"][ts(k, 128), :])
                nc.sync.dma_start(p["b0_s"][:, k:k + 1], p["b0"][ts(k, 128)])
                nc.sync.dma_start(p["b1_s"][:, k:k + 1], p["b1"][ts(k, 128)])
            for k in range(2):
                nc.sync.dma_start(p["w3t_s"][:, ts(k, 128)], p["w3t"][ts(k, 128), :])
                nc.sync.dma_start(p["b2_s"][:, k:k + 1], p["b2"][ts(k, 128)])
            nc.sync.dma_start(p["b3_s"][:, 0:1], p["b3"][:])
            for i in range(4):
                nc.gpsimd.memset(gtiles[i][:], 0.0)
                for j in range(NCH):
                    nc.gpsimd.memset(
                        fnt_tiles[i][:, j * 65 + 64:j * 65 + 65], 1.0)

            p["xg"] = ctx.enter_context(tc.tile_pool(name="xg", bufs=2))
            p["adj"] = ctx.enter_context(tc.tile_pool(name="adj", bufs=3))
            p["xs"] = ctx.enter_context(tc.tile_pool(name="xs", bufs=2))
            p["f1"] = ctx.enter_context(tc.tile_pool(name="f1", bufs=2))
            p["fn"] = ctx.enter_context(tc.tile_pool(name="fn", bufs=4))
            p["scr"] = ctx.enter_context(tc.tile_pool(name="scr", bufs=3))
            p["wwf"] = ctx.enter_context(tc.tile_pool(name="wwf", bufs=3))
            p["e"] = ctx.enter_context(tc.tile_pool(name="e", bufs=3))
            p["em"] = ctx.enter_context(tc.tile_pool(name="em", bufs=3))
            p["rz"] = ctx.enter_context(tc.tile_pool(name="rz", bufs=2))
            p["zsb"] = ctx.enter_context(tc.tile_pool(name="zsb", bufs=2))
            p["zb"] = ctx.enter_context(tc.tile_pool(name="zb", bufs=2))
            p["fnuf1"] = ctx.enter_context(tc.tile_pool(name="fnuf1", bufs=2))
            p["frow"] = ctx.enter_context(tc.tile_pool(name="frow", bufs=1))
            p["tiny"] = ctx.enter_context(tc.tile_pool(name="tiny", bufs=1))
            p["mlp"] = ctx.enter_context(tc.tile_pool(name="mlp", bufs=1))
            # PSUM: 4 (s_all) + 2 (prop/gate) + 2 (small) = 8 banks
            p["s_ps"] = ctx.enter_context(
                tc.tile_pool(name="s_ps", bufs=1, space="PSUM"))
            p["prop_ps"] = ctx.enter_context(
                tc.tile_pool(name="prop_ps", bufs=2, space="PSUM"))
            p["sm_ps"] = ctx.enter_context(
                tc.tile_pool(name="sm_ps", bufs=2, space="PSUM"))
            p["fnt_ps"] = p["sm_ps"]

            for gp in range(BPC // 2):
                pair = (2 * gp, 2 * gp + 1)
                adj_g, f1_g, fn_g = {}, {}, {}
                for g in pair:
                    ag = p["adj"].tile([128, NCH * N], BF16)
                    for j in range(NCH):
                        nc.sync.dma_start(ag[:, ts(j, N)],
                                          p["adjf"][g, ts(j, 128), :])
                    adj_g[g] = ag
                for i, g in enumerate(pair):
                    f1_g[g] = _emit_phase_a(nc, p, g, gtiles,
                                            fnt_tiles[2 * i:2 * i + 2])
                    fn_g[g] = f1_g[g]
                for t in range(1, T):
                    for i, g in enumerate(pair):
                        fnt_pair = fnt_tiles[2 * i:2 * i + 2]
                        fn_g[g] = _emit_iter(
                            nc, p, g, t, fn_g[g],
                            fnt_pair[(t - 1) % 2], fnt_pair[t % 2],
                            f1_g[g], adj_g[g], gtiles)

            _emit_norm_mlp(nc, p, gtiles)

    nc.compile()
    return nc


def _prep_inputs(inputs):
    bf = ml_dtypes.bfloat16
    x = np.asarray(inputs["node_attribute_matrix"], np.float32)
    adj = np.asarray(inputs["adjacent_matrix"])
    adjf = np.ascontiguousarray((adj != 0).astype(bf))
    xT = np.ascontiguousarray(x.transpose(0, 2, 1).astype(bf))  # [B, FEAT, N]

    common = {
        "wt": np.ascontiguousarray(np.asarray(inputs["W"], np.float32).T.astype(bf)),
        "wvt": np.ascontiguousarray(np.asarray(inputs["Wv"], np.float32).T.astype(bf)),
        "wwt": np.ascontiguousarray(np.asarray(inputs["Ww"], np.float32).T.astype(bf)),
        "wgt": np.ascontiguousarray(np.asarray(inputs["Wg"], np.float32).T.astype(bf)),
        "w0t": np.ascontiguousarray(np.asarray(inputs["W0"], np.float32).T),
        "w1t": np.ascontiguousarray(np.asarray(inputs["W1"], np.float32).T),
        "w2t": np.ascontiguousarray(np.asarray(inputs["W2"], np.float32).T),
        "w3t": np.ascontiguousarray(np.asarray(inputs["W3"], np.float32).T),
        "b0": np.asarray(inputs["b0"], np.float32),
        "b1": np.asarray(inputs["b1"], np.float32),
        "b2": np.asarray(inputs["b2"], np.float32),
        "b3": np.asarray(inputs["b3"], np.float32),
        "ident": np.eye(128, dtype=np.float32),
        "identb": np.eye(128).astype(bf),
    }
    in_maps = []
    for c in range(NCORES):
        sl = slice(c * BPC, (c + 1) * BPC)
        m = dict(common)
        m["xT"] = xT[sl]
        m["adjf"] = adjf[sl]
        in_maps.append(m)
    return in_maps


def kernel(**inputs) -> np.ndarray:
    if "nc" not in _STATE:
        _STATE["nc"] = _build_program()
    nc = _STATE["nc"]
    in_maps = _prep_inputs(inputs)

    trace = bool(int(os.environ.get("GNN_TRACE", "0")))
    kwargs = {}
    if trace:
        kwargs = dict(trace=True, tmpdir=os.environ.get("GNN_TRACE_DIR") or None)
    t0 = time.time()
    res = run_bass_kernel_spmd(nc, in_maps, list(range(NCORES)), **kwargs)
    _STATE["wall_s"] = time.time() - t0
    _STATE["exec_time_ns"] = res.exec_time_ns
    _STATE["results"] = res

    out = np.empty((B, 128), np.float32)
    for c in range(NCORES):
        out[c * BPC:(c + 1) * BPC] = res.results[c]["outT"].T
    return out


# revision 28
# speedup vs baseline: 1.8005x; 1.6955x over previous
"""GNN message-passing kernel for 8 Trainium2 NeuronCores.

Full (unsharded) inputs in, full output out. Data-parallel over the batch
dimension: 64 graphs -> 8 cores x 8 graphs. Parameters replicated.

Math per graph (reference semantics):
  X  = W @ x^T                          [EMB=128, N=512]
  F1 = sigmoid(Wv @ X)                  [RP=64, N=512]
  Fn = F1
  repeat 7x:
    WwF = Ww @ Fn                       [64, 512]
    S   = Fn^T(r-contract) WwF          [512, 512]   S[n,m] = sum_r Fn[r,n] WwF[r,m]
    A   = softmax_n(mask ? -inf : S)
    Fn  = (Fn @ A) * F1
  gates q=0..7: g_q = sum_n sigmoid(Wg @ Fn_q)       [64]
  fT = concat(gates); fT /= ||fT||; out = MLP(fT)    [128]

Device implementation notes:
  * loop tensors are bf16 (validated: output error stays ~1e-4); PSUM
    accumulation is fp32 throughout.
  * sigmoid(x) = 0.5 + 0.5*tanh(x/2)  -> tanh lives in the same ACT table set
    as exp, so no table switches in the main loop.
  * softmax without max-subtraction (S is bounded); multiplicative 0/1
    adjacency mask applied to exp(S) in one fused [128, 2048] DVE op (2x
    bf16 mode).  The four S chunks live in one 4-bank PSUM tile so exp is a
    single [128, 2048] ACT op.
  * column sums Z come from an appended ones-column in the propagation
    matmul lhsT; 1/Z via fast DVE reciprocal (input staged to SBUF - the
    custom op misreads PSUM re-reads); broadcast via gpsimd.
  * gate sums come from ACT accum_out, directly assembling the transposed
    feature matrix fTT [512, 8] used by the MLP.
  * rsqrt for the norm via bit-trick seed + Newton (no sqrt table needed).
"""

import os
import time

import ml_dtypes
import numpy as np

import concourse.bacc as bacc
import concourse.tile as tile
import concourse.mybir as mybir
from concourse.bass_utils import run_bass_kernel_spmd

F32 = mybir.dt.float32
BF16 = mybir.dt.bfloat16
U32 = mybir.dt.uint32
AF = mybir.ActivationFunctionType
ALU = mybir.AluOpType

B, N, FEAT, EMB, RP = 64, 512, 256, 128, 64
T = 8          # MAX_WALK_LEN (1 initial gate + 7 propagation steps)
NCORES = 8
BPC = B // NCORES   # graphs per core
NCH = N // 128      # n-chunks of 128
D0 = RP * T         # 512, MLP width

_STATE = {}


def ts(i, size):
    return slice(i * size, (i + 1) * size)


def _emit_phase_a(nc, p, g, gtiles, fnt_tiles):
    """X = W @ x^T, F1 = sigmoid(Wv X), gate 0, FnT(F1). Returns F1 tile."""
    xg = p["xg"].tile([128, 2 * N], BF16)
    for k in range(2):
        nc.sync.dma_start(xg[:, ts(k, N)], p["xT"][g, ts(k, 128), :])

    x_ps = p["sm_ps"].tile([128, N], F32, tag="sm", name="sm_t")
    for k in range(2):
        nc.tensor.matmul(x_ps[:], p["wt_s"][:, ts(k, 128)], xg[:, ts(k, N)],
                         start=(k == 0), stop=(k == 1))
    x_s = p["xs"].tile([128, N], BF16)
    nc.scalar.copy(x_s[:], x_ps[:])

    f1_ps = p["sm_ps"].tile([64, N], F32, tag="sm", name="sm_t")
    nc.tensor.matmul(f1_ps[:], p["wvt_s"][:], x_s[:], start=True, stop=True)
    scr = p["scr"].tile([64, N], F32)
    nc.scalar.activation(scr[:], f1_ps[:], AF.Tanh, scale=0.5)
    f1 = p["f1"].tile([64, N], BF16)
    nc.vector.tensor_scalar(f1[:], scr[:], 0.5, 0.5, ALU.mult, ALU.add)

    wwf_s = _emit_combined(nc, p, g, 0, f1, gtiles)
    _emit_fnt(nc, p, f1, fnt_tiles[0])
    return f1, wwf_s


def _emit_combined(nc, p, g, q, fn, gtiles):
    """One matmul computes both the gate pre-activation (rows 0:64, lhsT WgT)
    and the next iteration's WwF (rows 64:128, lhsT WwT).  gate_q =
    sum_n sigmoid(Wg Fn) = 256 + 0.5*sum_n tanh(0.5*(Wg Fn)); the raw
    tanh-sum accumulates into the transposed-feature assembly tile and the
    affine (0.5, +256) is applied in the normalization phase.
    Returns WwF in SBUF (bf16)."""
    cmm = p["prop_ps"].tile([128, N], F32, tag="pp", name="pp_t")
    nc.tensor.matmul(cmm[:], p["wgwwt_s"][:], fn[:], start=True, stop=True)
    scr = p["scr"].tile([64, N], F32)
    half = (q % 2) * 64
    acc = gtiles[q // 2][half:half + 64, g:g + 1]
    nc.scalar.activation(scr[:], cmm[0:64, :], AF.Tanh, scale=0.5,
                         accum_out=acc)
    wwf_s = p["wwf"].tile([64, N], BF16)
    nc.vector.tensor_copy(wwf_s[:], cmm[64:128, :])
    return wwf_s


def _emit_fnt(nc, p, fn, fnt):
    """PE-transpose Fn [64,512] into fnt chunks [128,64] (cols j*65..j*65+63).
    Column j*65+64 holds the persistent ones used for the Z row."""
    fnt_ps = p["fnt_ps"].tile([128, 4 * 64], BF16, tag="pp", name="pp_t")
    for j in range(NCH):
        nc.tensor.transpose(fnt_ps[:, ts(j, 64)], fn[:, ts(j, 128)],
                            p["identb_s"][0:64, 0:64])
    # one strided copy: chunks land at stride-65 offsets, skipping ones cols
    dst = fnt.rearrange("p (c k) -> p c k", k=65)[:, :, 0:64]
    src = fnt_ps[:].rearrange("p (c k) -> p c k", k=64)
    nc.vector.tensor_copy(dst, src)


def _emit_iter(nc, p, g, t, fn_prev, wwf_s, fnt_prev, fnt_next, f1, adj_g,
               gtiles):
    """One propagation step for graph g. Returns (new Fn, new WwF)."""
    # S in two 2-bank halves so exp/mask/prop pipeline at half-iter grain
    prop = p["prop_ps"].tile([65, N], F32, tag="pp", name="pp_t")
    for h in range(2):
        s_h = p["s_ps"].tile([128, 2 * N], F32, tag="s", name="s_t")
        for j in range(2):
            nc.tensor.matmul(s_h[:, ts(j, N)],
                             fn_prev[:, ts(2 * h + j, 128)], wwf_s[:],
                             start=True, stop=True)
        e_h = p["e"].tile([128, 2 * N], BF16)
        nc.scalar.activation(e_h[:], s_h[:], AF.Exp)
        em_h = p["em"].tile([128, 2 * N], BF16)
        nc.vector.tensor_tensor(em_h[:], e_h[:],
                                adj_g[:, ts(h, 2 * N)], ALU.mult)
        for j in range(2):
            jj = 2 * h + j
            nc.tensor.matmul(prop[:], fnt_prev[:, jj * 65:(jj + 1) * 65],
                             em_h[:, ts(j, N)], start=(jj == 0),
                             stop=(jj == 3), skip_group_check=True)

    # softmax normalization: rz = 1/Z, broadcast across partitions via gpsimd
    zsb = p["zsb"].tile([1, N], F32)
    nc.scalar.copy(zsb[:], prop[64:65, :])
    rz = p["rz"].tile([1, N], F32)
    nc.vector.reciprocal_approx_fast(rz[:], zsb[:])
    zb = p["zb"].tile([64, N], F32)
    nc.gpsimd.partition_broadcast(zb[:], rz[:])
    fnuf1 = p["fnuf1"].tile([64, N], F32)
    nc.vector.tensor_tensor(fnuf1[:], prop[0:64, :], f1[:], ALU.mult)
    fn_new = p["fn"].tile([64, N], BF16)
    nc.vector.tensor_tensor(fn_new[:], fnuf1[:], zb[:], ALU.mult)

    wwf_new = _emit_combined(nc, p, g, t, fn_new, gtiles)
    if t < T - 1:
        _emit_fnt(nc, p, fn_new, fnt_next)
    return fn_new, wwf_new


def _emit_norm_mlp(nc, p, gtiles):
    """Gate affine + L2 normalization + 4-layer MLP, all graphs at once."""
    ident = p["ident_s"]
    # transpose the 4 assembly tiles [128, BPC] -> row layout [BPC, 512]
    row_ps = p["sm_ps"].tile([BPC, N], F32, tag="sm", name="sm_t")
    for j in range(NCH):
        nc.tensor.transpose(row_ps[:, ts(j, 128)], gtiles[j][:, 0:BPC],
                            ident[:, 0:128])
    f_row = p["frow"].tile([BPC, N], F32)
    # gate = 0.5*acc + 256 applied during the PSUM->SBUF copy
    nc.vector.tensor_scalar(f_row[:], row_ps[:], 0.5, 256.0, ALU.mult, ALU.add)

    # ss[g] = sum_k f_row[g,k]^2 via Square activation with accumulate
    sq = p["frow"].tile([BPC, N], F32)
    ss = p["tiny"].tile([BPC, 1], F32, tag="ss")
    nc.scalar.activation(sq[:], f_row[:], AF.Square, accum_out=ss[:])

    # rn = rsqrt(ss): bit-trick sqrt seed, fast reciprocal, 3 Newton steps
    tmp = p["tiny"].tile([BPC, 1], F32, tag="t0")
    y = p["tiny"].tile([BPC, 1], F32, tag="t1")
    a = p["tiny"].tile([BPC, 1], F32, tag="t2")
    nc.vector.tensor_scalar(tmp[:].bitcast(U32), ss[:].bitcast(U32),
                            1, None, ALU.logical_shift_right)
    nc.vector.tensor_scalar(tmp[:].bitcast(U32), tmp[:].bitcast(U32),
                            0x1FBD1DF5, None, ALU.add)
    nc.vector.reciprocal_approx_fast(y[:], tmp[:])
    for _ in range(3):
        nc.vector.tensor_tensor(a[:], y[:], y[:], ALU.mult)       # y^2
        nc.vector.tensor_tensor(a[:], ss[:], a[:], ALU.mult)      # ss*y^2
        nc.vector.tensor_scalar(a[:], a[:], -0.5, 1.5, ALU.mult, ALU.add)
        nc.vector.tensor_tensor(y[:], y[:], a[:], ALU.mult)

    fn_row = p["frow"].tile([BPC, N], F32)
    nc.vector.tensor_scalar(fn_row[:], f_row[:], y[:], None, ALU.mult)

    # back to transposed layout [512, BPC] for the MLP
    h0 = p["mlp"].tile([128, 4 * BPC], F32, tag="h0")
    for j in range(NCH):
        t_ps = p["sm_ps"].tile([128, BPC], F32, tag="sm", name="sm_t")
        nc.tensor.transpose(t_ps[:, 0:BPC], fn_row[:, ts(j, 128)],
                            ident[0:BPC, 0:BPC])
        nc.vector.tensor_copy(h0[:, ts(j, BPC)], t_ps[:, 0:BPC])

    # MLP in transposed layout: h_next[j,g] = act(sum_k WT[k,j] h[k,g] + b[j])
    def layer(h_in, kch, jch, w_s, b_s, act, tag):
        h_out = p["mlp"].tile([128, jch * BPC], F32, tag=tag)
        for j in range(jch):
            mm = p["sm_ps"].tile([128, BPC], F32, tag="sm", name="sm_t")
            for k in range(kch):
                nc.tensor.matmul(mm[:], w_s[:, k * (jch * 128) + j * 128:
                                             k * (jch * 128) + (j + 1) * 128],
                                 h_in[:, ts(k, BPC)],
                                 start=(k == 0), stop=(k == kch - 1))
            nc.scalar.activation(h_out[:, ts(j, BPC)], mm[:], act,
                                 bias=b_s[:, j:j + 1])
        return h_out

    h1 = layer(h0, 4, 4, p["w0t_s"], p["b0_s"], AF.Relu, "h1")
    h2 = layer(h1, 4, 4, p["w1t_s"], p["b1_s"], AF.Relu, "h2")
    h3 = layer(h2, 4, 2, p["w2t_s"], p["b2_s"], AF.Relu, "h3")
    h4 = layer(h3, 2, 1, p["w3t_s"], p["b3_s"], AF.Identity, "h4")
    nc.sync.dma_start(p["outT"][:, :], h4[:, 0:BPC])


def _build_program():
    nc = bacc.Bacc("TRN2", target_bir_lowering=False, debug=False,
                   num_devices=NCORES)
    p = {}
    p["xT"] = nc.dram_tensor("xT", [BPC, FEAT, N], BF16, kind="ExternalInput").ap()
    p["adjf"] = nc.dram_tensor("adjf", [BPC, N, N], BF16, kind="ExternalInput").ap()
    for name, shape, dt in [("wt", [FEAT, EMB], BF16), ("wvt", [EMB, RP], BF16),
                            ("wgwwt", [RP, 2 * RP], BF16),
                            ("w0t", [D0, D0], F32), ("w1t", [D0, D0], F32),
                            ("w2t", [D0, D0 // 2], F32),
                            ("w3t", [D0 // 2, 128], F32),
                            ("b0", [D0], F32), ("b1", [D0], F32),
                            ("b2", [D0 // 2], F32), ("b3", [128], F32),
                            ("ident", [128, 128], F32),
                            ("identb", [128, 128], BF16)]:
        p[name] = nc.dram_tensor(name, shape, dt, kind="ExternalInput").ap()
    p["outT"] = nc.dram_tensor("outT", [128, BPC], F32, kind="ExternalOutput").ap()

    with tile.TileContext(nc) as tc:
        import contextlib
        with contextlib.ExitStack() as ctx:
            const = ctx.enter_context(tc.tile_pool(name="const", bufs=1))
            p["wt_s"] = const.tile([128, 2 * 128], BF16, tag="wt", name="wt_s")
            p["wvt_s"] = const.tile([128, 64], BF16, tag="wvt", name="wvt_s")
            p["wgwwt_s"] = const.tile([64, 128], BF16, tag="wgwwt",
                                      name="wgwwt_s")
            p["ident_s"] = const.tile([128, 128], F32, tag="ident", name="ident_s")
            p["identb_s"] = const.tile([128, 128], BF16, tag="identb",
                                       name="identb_s")
            p["w0t_s"] = const.tile([128, 4 * D0], F32, tag="w0t", name="w0t_s")
            p["w1t_s"] = const.tile([128, 4 * D0], F32, tag="w1t", name="w1t_s")
            p["w2t_s"] = const.tile([128, 4 * (D0 // 2)], F32, tag="w2t",
                                    name="w2t_s")
            p["w3t_s"] = const.tile([128, 2 * 128], F32, tag="w3t", name="w3t_s")
            p["b0_s"] = const.tile([128, 4], F32, tag="b0", name="b0_s")
            p["b1_s"] = const.tile([128, 4], F32, tag="b1", name="b1_s")
            p["b2_s"] = const.tile([128, 2], F32, tag="b2", name="b2_s")
            p["b3_s"] = const.tile([128, 1], F32, tag="b3", name="b3_s")
            gtiles = [const.tile([128, BPC], F32, tag=f"gt{i}", name=f"gt{i}")
                      for i in range(4)]
            fnt_tiles = [const.tile([128, 4 * 65], BF16, tag=f"fnt{i}",
                                    name=f"fnt{i}")
                         for i in range(4)]  # 2 ping-pong per graph of a pair

            for k in range(2):
                nc.sync.dma_start(p["wt_s"][:, ts(k, 128)], p["wt"][ts(k, 128), :])
            nc.sync.dma_start(p["wvt_s"][:], p["wvt"][:, :])
            nc.sync.dma_start(p["wgwwt_s"][:], p["wgwwt"][:, :])
            nc.sync.dma_start(p["ident_s"][:], p["ident"][:, :])
            nc.sync.dma_start(p["identb_s"][:], p["identb"][:, :])
            for k in range(4):
                nc.sync.dma_start(p["w0t_s"][:, ts(k, D0)], p["w0t"][ts(k, 128), :])
                nc.sync.dma_start(p["w1t_s"][:, ts(k, D0)], p["w1t"][ts(k, 128), :])
                nc.sync.dma_start(p["w2t_s"][:, ts(k, D0 // 2)],
                                  p["w2t"][ts(k, 128), :])
                nc.sync.dma_start(p["b0_s"][:, k:k + 1], p["b0"][ts(k, 128)])
                nc.sync.dma_start(p["b1_s"][:, k:k + 1], p["b1"][ts(k, 128)])
            for k in range(2):
                nc.sync.dma_start(p["w3t_s"][:, ts(k, 128)], p["w3t"][ts(k, 128), :])
                nc.sync.dma_start(p["b2_s"][:, k:k + 1], p["b2"][ts(k, 128)])
            nc.sync.dma_start(p["b3_s"][:, 0:1], p["b3"][:])
            for i in range(4):
                nc.gpsimd.memset(gtiles[i][:], 0.0)
                for j in range(NCH):
                    nc.gpsimd.memset(
                        fnt_tiles[i][:, j * 65 + 64:j * 65 + 65], 1.0)

            p["xg"] = ctx.enter_context(tc.tile_pool(name="xg", bufs=2))
            p["adj"] = ctx.enter_context(tc.tile_pool(name="adj", bufs=3))
            p["xs"] = ctx.enter_context(tc.tile_pool(name="xs", bufs=2))
            p["f1"] = ctx.enter_context(tc.tile_pool(name="f1", bufs=2))
            p["fn"] = ctx.enter_context(tc.tile_pool(name="fn", bufs=4))
            p["scr"] = ctx.enter_context(tc.tile_pool(name="scr", bufs=3))
            p["wwf"] = ctx.enter_context(tc.tile_pool(name="wwf", bufs=3))
            p["e"] = ctx.enter_context(tc.tile_pool(name="e", bufs=3))
            p["em"] = ctx.enter_context(tc.tile_pool(name="em", bufs=3))
            p["rz"] = ctx.enter_context(tc.tile_pool(name="rz", bufs=2))
            p["zsb"] = ctx.enter_context(tc.tile_pool(name="zsb", bufs=2))
            p["zb"] = ctx.enter_context(tc.tile_pool(name="zb", bufs=2))
            p["fnuf1"] = ctx.enter_context(tc.tile_pool(name="fnuf1", bufs=2))
            p["frow"] = ctx.enter_context(tc.tile_pool(name="frow", bufs=1))
            p["tiny"] = ctx.enter_context(tc.tile_pool(name="tiny", bufs=1))
            p["mlp"] = ctx.enter_context(tc.tile_pool(name="mlp", bufs=1))
            # PSUM: 2x2 (S halves) + 3 (prop/combined/fnt) + 1 (misc) = 8
            p["s_ps"] = ctx.enter_context(
                tc.tile_pool(name="s_ps", bufs=2, space="PSUM"))
            p["prop_ps"] = ctx.enter_context(
                tc.tile_pool(name="prop_ps", bufs=3, space="PSUM"))
            p["sm_ps"] = ctx.enter_context(
                tc.tile_pool(name="sm_ps", bufs=1, space="PSUM"))
            p["fnt_ps"] = p["prop_ps"]

            for gp in range(BPC // 2):
                pair = (2 * gp, 2 * gp + 1)
                adj_g, f1_g, fn_g = {}, {}, {}
                for g in pair:
                    ag = p["adj"].tile([128, NCH * N], BF16)
                    for j in range(NCH):
                        nc.sync.dma_start(ag[:, ts(j, N)],
                                          p["adjf"][g, ts(j, 128), :])
                    adj_g[g] = ag
                wwf_g = {}
                for i, g in enumerate(pair):
                    f1_g[g], wwf_g[g] = _emit_phase_a(
                        nc, p, g, gtiles, fnt_tiles[2 * i:2 * i + 2])
                    fn_g[g] = f1_g[g]
                for t in range(1, T):
                    for i, g in enumerate(pair):
                        fnt_pair = fnt_tiles[2 * i:2 * i + 2]
                        fn_g[g], wwf_g[g] = _emit_iter(
                            nc, p, g, t, fn_g[g], wwf_g[g],
                            fnt_pair[(t - 1) % 2], fnt_pair[t % 2],
                            f1_g[g], adj_g[g], gtiles)

            _emit_norm_mlp(nc, p, gtiles)

    nc.compile()
    return nc


def _prep_inputs(inputs):
    bf = ml_dtypes.bfloat16
    x = np.asarray(inputs["node_attribute_matrix"], np.float32)
    adj = np.asarray(inputs["adjacent_matrix"])
    adjf = np.ascontiguousarray((adj != 0).astype(bf))
    xT = np.ascontiguousarray(x.transpose(0, 2, 1).astype(bf))  # [B, FEAT, N]

    common = {
        "wt": np.ascontiguousarray(np.asarray(inputs["W"], np.float32).T.astype(bf)),
        "wvt": np.ascontiguousarray(np.asarray(inputs["Wv"], np.float32).T.astype(bf)),
        "wgwwt": np.ascontiguousarray(np.hstack([
            np.asarray(inputs["Wg"], np.float32).T,
            np.asarray(inputs["Ww"], np.float32).T]).astype(bf)),
        "w0t": np.ascontiguousarray(np.asarray(inputs["W0"], np.float32).T),
        "w1t": np.ascontiguousarray(np.asarray(inputs["W1"], np.float32).T),
        "w2t": np.ascontiguousarray(np.asarray(inputs["W2"], np.float32).T),
        "w3t": np.ascontiguousarray(np.asarray(inputs["W3"], np.float32).T),
        "b0": np.asarray(inputs["b0"], np.float32),
        "b1": np.asarray(inputs["b1"], np.float32),
        "b2": np.asarray(inputs["b2"], np.float32),
        "b3": np.asarray(inputs["b3"], np.float32),
        "ident": np.eye(128, dtype=np.float32),
        "identb": np.eye(128).astype(bf),
    }
    in_maps = []
    for c in range(NCORES):
        sl = slice(c * BPC, (c + 1) * BPC)
        m = dict(common)
        m["xT"] = xT[sl]
        m["adjf"] = adjf[sl]
        in_maps.append(m)
    return in_maps


def kernel(**inputs) -> np.ndarray:
    if "nc" not in _STATE:
        _STATE["nc"] = _build_program()
    nc = _STATE["nc"]
    in_maps = _prep_inputs(inputs)

    trace = bool(int(os.environ.get("GNN_TRACE", "0")))
    kwargs = {}
    if trace:
        kwargs = dict(trace=True, tmpdir=os.environ.get("GNN_TRACE_DIR") or None)
    t0 = time.time()
    res = run_bass_kernel_spmd(nc, in_maps, list(range(NCORES)), **kwargs)
    _STATE["wall_s"] = time.time() - t0
    _STATE["exec_time_ns"] = res.exec_time_ns
    _STATE["results"] = res

    out = np.empty((B, 128), np.float32)
    for c in range(NCORES):
        out[c * BPC:(c + 1) * BPC] = res.results[c]["outT"].T
    return out


# revision 32
# speedup vs baseline: 1.8558x; 1.0307x over previous
"""GNN message-passing kernel for 8 Trainium2 NeuronCores.

Full (unsharded) inputs in, full output out. Data-parallel over the batch
dimension: 64 graphs -> 8 cores x 8 graphs. Parameters replicated.

Math per graph (reference semantics):
  X  = W @ x^T                          [EMB=128, N=512]
  F1 = sigmoid(Wv @ X)                  [RP=64, N=512]
  Fn = F1
  repeat 7x:
    WwF = Ww @ Fn                       [64, 512]
    S   = Fn^T(r-contract) WwF          [512, 512]   S[n,m] = sum_r Fn[r,n] WwF[r,m]
    A   = softmax_n(mask ? -inf : S)
    Fn  = (Fn @ A) * F1
  gates q=0..7: g_q = sum_n sigmoid(Wg @ Fn_q)       [64]
  fT = concat(gates); fT /= ||fT||; out = MLP(fT)    [128]

Device implementation notes:
  * loop tensors are bf16 (validated: output error stays ~1e-4); PSUM
    accumulation is fp32 throughout.
  * sigmoid(x) = 0.5 + 0.5*tanh(x/2)  -> tanh lives in the same ACT table set
    as exp, so no table switches in the main loop.
  * softmax without max-subtraction (S is bounded); multiplicative 0/1
    adjacency mask applied to exp(S) in one fused [128, 2048] DVE op (2x
    bf16 mode).  The four S chunks live in one 4-bank PSUM tile so exp is a
    single [128, 2048] ACT op.
  * column sums Z come from an appended ones-column in the propagation
    matmul lhsT; 1/Z via fast DVE reciprocal (input staged to SBUF - the
    custom op misreads PSUM re-reads); broadcast via gpsimd.
  * gate sums come from ACT accum_out, directly assembling the transposed
    feature matrix fTT [512, 8] used by the MLP.
  * rsqrt for the norm via bit-trick seed + Newton (no sqrt table needed).
"""

import os
import time

import ml_dtypes
import numpy as np

import concourse.bacc as bacc
import concourse.tile as tile
import concourse.mybir as mybir
from concourse.bass_utils import run_bass_kernel_spmd

F32 = mybir.dt.float32
BF16 = mybir.dt.bfloat16
U32 = mybir.dt.uint32
AF = mybir.ActivationFunctionType
ALU = mybir.AluOpType

B, N, FEAT, EMB, RP = 64, 512, 256, 128, 64
T = 8          # MAX_WALK_LEN (1 initial gate + 7 propagation steps)
NCORES = 8
BPC = B // NCORES   # graphs per core
NCH = N // 128      # n-chunks of 128
D0 = RP * T         # 512, MLP width

_STATE = {}


def ts(i, size):
    return slice(i * size, (i + 1) * size)


def _emit_phase_a(nc, p, g, gtiles, fnt_tiles):
    """X = W @ x^T, F1 = sigmoid(Wv X), gate 0, FnT(F1). Returns F1 tile."""
    xg = p["xg"].tile([128, 2 * N], BF16)
    for k in range(2):
        nc.sync.dma_start(xg[:, ts(k, N)], p["xT"][g, ts(k, 128), :])

    x_ps = p["sm_ps"].tile([128, N], F32, tag="pp", name="pp_t")
    for k in range(2):
        nc.tensor.matmul(x_ps[:], p["wt_s"][:, ts(k, 128)], xg[:, ts(k, N)],
                         start=(k == 0), stop=(k == 1))
    x_s = p["xs"].tile([128, N], BF16)
    nc.scalar.copy(x_s[:], x_ps[:])

    f1_ps = p["sm_ps"].tile([64, N], F32, tag="pp", name="pp_t")
    nc.tensor.matmul(f1_ps[:], p["wvt_s"][:], x_s[:], start=True, stop=True)
    scr = p["scr"].tile([64, N], F32)
    nc.scalar.activation(scr[:], f1_ps[:], AF.Tanh, scale=0.5)
    f1 = p["f1"].tile([64, N], BF16)
    nc.vector.tensor_scalar(f1[:], scr[:], 0.5, 0.5, ALU.mult, ALU.add)

    wwf_s = _emit_combined(nc, p, g, 0, f1, gtiles)
    _emit_fnt(nc, p, f1, fnt_tiles[0])
    return f1, wwf_s


def _emit_combined(nc, p, g, q, fn, gtiles):
    """One matmul computes both the gate pre-activation (rows 0:64, lhsT WgT)
    and the next iteration's WwF (rows 64:128, lhsT WwT).  gate_q =
    sum_n sigmoid(Wg Fn) = 256 + 0.5*sum_n tanh(0.5*(Wg Fn)); the raw
    tanh-sum accumulates into the transposed-feature assembly tile and the
    affine (0.5, +256) is applied in the normalization phase.
    Returns WwF in SBUF (bf16)."""
    cmm = p["prop_ps"].tile([128, N], F32, tag="pp", name="pp_t")
    nc.tensor.matmul(cmm[:], p["wgwwt_s"][:], fn[:], start=True, stop=True)
    scr = p["scr"].tile([64, N], F32)
    half = (q % 2) * 64
    acc = gtiles[q // 2][half:half + 64, g:g + 1]
    nc.scalar.activation(scr[:], cmm[0:64, :], AF.Tanh, scale=0.5,
                         accum_out=acc)
    wwf_s = p["wwf"].tile([64, N], BF16)
    nc.vector.tensor_copy(wwf_s[:], cmm[64:128, :])
    return wwf_s


def _emit_fnt(nc, p, fn, fnt):
    """PE-transpose Fn [64,512] into fnt chunks [128,64] (cols j*65..j*65+63).
    Column j*65+64 holds the persistent ones used for the Z row."""
    fnt_ps = p["fnt_ps"].tile([128, 4 * 64], BF16, tag="pp", name="pp_t")
    for j in range(NCH):
        nc.tensor.transpose(fnt_ps[:, ts(j, 64)], fn[:, ts(j, 128)],
                            p["identb_s"][0:64, 0:64])
    # one strided copy: chunks land at stride-65 offsets, skipping ones cols
    dst = fnt.rearrange("p (c k) -> p c k", k=65)[:, :, 0:64]
    src = fnt_ps[:].rearrange("p (c k) -> p c k", k=64)
    nc.vector.tensor_copy(dst, src)


def _emit_iter_head(nc, p, fn_prev, wwf_s, adj_g, h):
    """S-chunk half h -> exp -> mask. Returns masked exp tile [128, 1024]."""
    s_h = p["s_ps"].tile([128, 2 * N], F32, tag="s", name="s_t")
    for j in range(2):
        nc.tensor.matmul(s_h[:, ts(j, N)],
                         fn_prev[:, ts(2 * h + j, 128)], wwf_s[:],
                         start=True, stop=True)
    e_h = p["e"].tile([128, 2 * N], BF16)
    nc.scalar.activation(e_h[:], s_h[:], AF.Exp)
    em_h = p["em"].tile([128, 2 * N], BF16)
    nc.vector.tensor_tensor(em_h[:], e_h[:],
                            adj_g[:, ts(h, 2 * N)], ALU.mult)
    return em_h


def _emit_iter_tail(nc, p, g, t, prop, fnt_next, f1, gtiles):
    """Softmax normalization + Fn update + gate/WwF + FnT transposes."""
    zsb = p["zsb"].tile([1, N], F32)
    nc.scalar.copy(zsb[:], prop[64:65, :])
    rz = p["rz"].tile([1, N], F32)
    nc.vector.reciprocal_approx_fast(rz[:], zsb[:])
    zb = p["zb"].tile([64, N], F32)
    nc.gpsimd.partition_broadcast(zb[:], rz[:])
    fnuf1 = p["fnuf1"].tile([64, N], F32)
    nc.vector.tensor_tensor(fnuf1[:], prop[0:64, :], f1[:], ALU.mult)
    fn_new = p["fn"].tile([64, N], BF16)
    nc.vector.tensor_tensor(fn_new[:], fnuf1[:], zb[:], ALU.mult)

    wwf_new = _emit_combined(nc, p, g, t, fn_new, gtiles)
    if t < T - 1:
        _emit_fnt(nc, p, fn_new, fnt_next)
    return fn_new, wwf_new


def _emit_norm_mlp(nc, p, gtiles):
    """Gate affine + L2 normalization + 4-layer MLP, all graphs at once."""
    ident = p["ident_s"]
    # transpose the 4 assembly tiles [128, BPC] -> row layout [BPC, 512]
    row_ps = p["sm_ps"].tile([BPC, N], F32, tag="pp", name="pp_t")
    for j in range(NCH):
        nc.tensor.transpose(row_ps[:, ts(j, 128)], gtiles[j][:, 0:BPC],
                            ident[:, 0:128])
    f_row = p["frow"].tile([BPC, N], F32)
    # gate = 0.5*acc + 256 applied during the PSUM->SBUF copy
    nc.vector.tensor_scalar(f_row[:], row_ps[:], 0.5, 256.0, ALU.mult, ALU.add)

    # ss[g] = sum_k f_row[g,k]^2 via Square activation with accumulate
    sq = p["frow"].tile([BPC, N], F32)
    ss = p["tiny"].tile([BPC, 1], F32, tag="ss")
    nc.scalar.activation(sq[:], f_row[:], AF.Square, accum_out=ss[:])

    # rn = rsqrt(ss): bit-trick sqrt seed, fast reciprocal, 3 Newton steps
    tmp = p["tiny"].tile([BPC, 1], F32, tag="t0")
    y = p["tiny"].tile([BPC, 1], F32, tag="t1")
    a = p["tiny"].tile([BPC, 1], F32, tag="t2")
    nc.vector.tensor_scalar(tmp[:].bitcast(U32), ss[:].bitcast(U32),
                            1, None, ALU.logical_shift_right)
    nc.vector.tensor_scalar(tmp[:].bitcast(U32), tmp[:].bitcast(U32),
                            0x1FBD1DF5, None, ALU.add)
    nc.vector.reciprocal_approx_fast(y[:], tmp[:])
    for _ in range(3):
        nc.vector.tensor_tensor(a[:], y[:], y[:], ALU.mult)       # y^2
        nc.vector.tensor_tensor(a[:], ss[:], a[:], ALU.mult)      # ss*y^2
        nc.vector.tensor_scalar(a[:], a[:], -0.5, 1.5, ALU.mult, ALU.add)
        nc.vector.tensor_tensor(y[:], y[:], a[:], ALU.mult)

    fn_row = p["frow"].tile([BPC, N], F32)
    nc.vector.tensor_scalar(fn_row[:], f_row[:], y[:], None, ALU.mult)

    # back to transposed layout [512, BPC] for the MLP
    h0 = p["mlp"].tile([128, 4 * BPC], F32, tag="h0")
    for j in range(NCH):
        t_ps = p["sm_ps"].tile([128, BPC], F32, tag="pp", name="pp_t")
        nc.tensor.transpose(t_ps[:, 0:BPC], fn_row[:, ts(j, 128)],
                            ident[0:BPC, 0:BPC])
        nc.vector.tensor_copy(h0[:, ts(j, BPC)], t_ps[:, 0:BPC])

    # MLP in transposed layout: h_next[j,g] = act(sum_k WT[k,j] h[k,g] + b[j])
    def layer(h_in, kch, jch, w_s, b_s, act, tag):
        h_out = p["mlp"].tile([128, jch * BPC], F32, tag=tag)
        for j in range(jch):
            mm = p["sm_ps"].tile([128, BPC], F32, tag="pp", name="pp_t")
            for k in range(kch):
                nc.tensor.matmul(mm[:], w_s[:, k * (jch * 128) + j * 128:
                                             k * (jch * 128) + (j + 1) * 128],
                                 h_in[:, ts(k, BPC)],
                                 start=(k == 0), stop=(k == kch - 1))
            nc.scalar.activation(h_out[:, ts(j, BPC)], mm[:], act,
                                 bias=b_s[:, j:j + 1])
        return h_out

    h1 = layer(h0, 4, 4, p["w0t_s"], p["b0_s"], AF.Relu, "h1")
    h2 = layer(h1, 4, 4, p["w1t_s"], p["b1_s"], AF.Relu, "h2")
    h3 = layer(h2, 4, 2, p["w2t_s"], p["b2_s"], AF.Relu, "h3")
    h4 = layer(h3, 2, 1, p["w3t_s"], p["b3_s"], AF.Identity, "h4")
    nc.sync.dma_start(p["outT"][:, :], h4[:, 0:BPC])


def _build_program():
    nc = bacc.Bacc("TRN2", target_bir_lowering=False, debug=False,
                   num_devices=NCORES)
    p = {}
    p["xT"] = nc.dram_tensor("xT", [BPC, FEAT, N], BF16, kind="ExternalInput").ap()
    p["adjf"] = nc.dram_tensor("adjf", [BPC, N, N], BF16, kind="ExternalInput").ap()
    for name, shape, dt in [("wt", [FEAT, EMB], BF16), ("wvt", [EMB, RP], BF16),
                            ("wgwwt", [RP, 2 * RP], BF16),
                            ("w0t", [D0, D0], F32), ("w1t", [D0, D0], F32),
                            ("w2t", [D0, D0 // 2], F32),
                            ("w3t", [D0 // 2, 128], F32),
                            ("b0", [D0], F32), ("b1", [D0], F32),
                            ("b2", [D0 // 2], F32), ("b3", [128], F32),
                            ("ident", [128, 128], F32),
                            ("identb", [128, 128], BF16)]:
        p[name] = nc.dram_tensor(name, shape, dt, kind="ExternalInput").ap()
    p["outT"] = nc.dram_tensor("outT", [128, BPC], F32, kind="ExternalOutput").ap()

    with tile.TileContext(nc) as tc:
        import contextlib
        with contextlib.ExitStack() as ctx:
            const = ctx.enter_context(tc.tile_pool(name="const", bufs=1))
            p["wt_s"] = const.tile([128, 2 * 128], BF16, tag="wt", name="wt_s")
            p["wvt_s"] = const.tile([128, 64], BF16, tag="wvt", name="wvt_s")
            p["wgwwt_s"] = const.tile([64, 128], BF16, tag="wgwwt",
                                      name="wgwwt_s")
            p["ident_s"] = const.tile([128, 128], F32, tag="ident", name="ident_s")
            p["identb_s"] = const.tile([128, 128], BF16, tag="identb",
                                       name="identb_s")
            p["w0t_s"] = const.tile([128, 4 * D0], F32, tag="w0t", name="w0t_s")
            p["w1t_s"] = const.tile([128, 4 * D0], F32, tag="w1t", name="w1t_s")
            p["w2t_s"] = const.tile([128, 4 * (D0 // 2)], F32, tag="w2t",
                                    name="w2t_s")
            p["w3t_s"] = const.tile([128, 2 * 128], F32, tag="w3t", name="w3t_s")
            p["b0_s"] = const.tile([128, 4], F32, tag="b0", name="b0_s")
            p["b1_s"] = const.tile([128, 4], F32, tag="b1", name="b1_s")
            p["b2_s"] = const.tile([128, 2], F32, tag="b2", name="b2_s")
            p["b3_s"] = const.tile([128, 1], F32, tag="b3", name="b3_s")
            gtiles = [const.tile([128, BPC], F32, tag=f"gt{i}", name=f"gt{i}")
                      for i in range(4)]
            fnt_tiles = [const.tile([128, 4 * 65], BF16, tag=f"fnt{i}",
                                    name=f"fnt{i}")
                         for i in range(8)]  # 2 ping-pong x 2 graphs x 2 pairs

            for k in range(2):
                nc.sync.dma_start(p["wt_s"][:, ts(k, 128)], p["wt"][ts(k, 128), :])
            nc.sync.dma_start(p["wvt_s"][:], p["wvt"][:, :])
            nc.sync.dma_start(p["wgwwt_s"][:], p["wgwwt"][:, :])
            nc.sync.dma_start(p["ident_s"][:], p["ident"][:, :])
            nc.sync.dma_start(p["identb_s"][:], p["identb"][:, :])
            for i in range(4):
                nc.gpsimd.memset(gtiles[i][:], 0.0)
            for i in range(8):
                for j in range(NCH):
                    nc.gpsimd.memset(
                        fnt_tiles[i][:, j * 65 + 64:j * 65 + 65], 1.0)

            p["xg"] = ctx.enter_context(tc.tile_pool(name="xg", bufs=2))
            p["adj"] = ctx.enter_context(tc.tile_pool(name="adj", bufs=3))
            p["xs"] = ctx.enter_context(tc.tile_pool(name="xs", bufs=2))
            p["f1"] = ctx.enter_context(tc.tile_pool(name="f1", bufs=2))
            p["fn"] = ctx.enter_context(tc.tile_pool(name="fn", bufs=4))
            p["scr"] = ctx.enter_context(tc.tile_pool(name="scr", bufs=3))
            p["wwf"] = ctx.enter_context(tc.tile_pool(name="wwf", bufs=3))
            p["e"] = ctx.enter_context(tc.tile_pool(name="e", bufs=3))
            p["em"] = ctx.enter_context(tc.tile_pool(name="em", bufs=3))
            p["rz"] = ctx.enter_context(tc.tile_pool(name="rz", bufs=2))
            p["zsb"] = ctx.enter_context(tc.tile_pool(name="zsb", bufs=2))
            p["zb"] = ctx.enter_context(tc.tile_pool(name="zb", bufs=2))
            p["fnuf1"] = ctx.enter_context(tc.tile_pool(name="fnuf1", bufs=2))
            p["frow"] = ctx.enter_context(tc.tile_pool(name="frow", bufs=1))
            p["tiny"] = ctx.enter_context(tc.tile_pool(name="tiny", bufs=1))
            p["mlp"] = ctx.enter_context(tc.tile_pool(name="mlp", bufs=1))
            # PSUM: 2x2 (S halves) + 3 (prop/combined/fnt) + 1 (misc) = 8
            p["s_ps"] = ctx.enter_context(
                tc.tile_pool(name="s_ps", bufs=2, space="PSUM"))
            p["prop_ps"] = ctx.enter_context(
                tc.tile_pool(name="prop_ps", bufs=4, space="PSUM"))
            p["sm_ps"] = p["prop_ps"]
            p["fnt_ps"] = p["prop_ps"]

            for gp in range(BPC // 2):
                pair = (2 * gp, 2 * gp + 1)
                fofs = 4 * (gp % 2)
                adj_g, f1_g, fn_g = {}, {}, {}
                for g in pair:
                    ag = p["adj"].tile([128, NCH * N], BF16)
                    for j in range(NCH):
                        nc.sync.dma_start(ag[:, ts(j, N)],
                                          p["adjf"][g, ts(j, 128), :])
                    adj_g[g] = ag
                wwf_g = {}
                for i, g in enumerate(pair):
                    f1_g[g], wwf_g[g] = _emit_phase_a(
                        nc, p, g, gtiles,
                        fnt_tiles[fofs + 2 * i:fofs + 2 * i + 2])
                    fn_g[g] = f1_g[g]
                if gp == 0:
                    # MLP weights are only needed at the very end; loading
                    # them here keeps the startup DMAs for pair 0 unblocked
                    for k in range(4):
                        nc.sync.dma_start(p["w0t_s"][:, ts(k, D0)],
                                          p["w0t"][ts(k, 128), :])
                        nc.sync.dma_start(p["w1t_s"][:, ts(k, D0)],
                                          p["w1t"][ts(k, 128), :])
                        nc.sync.dma_start(p["w2t_s"][:, ts(k, D0 // 2)],
                                          p["w2t"][ts(k, 128), :])
                        nc.sync.dma_start(p["b0_s"][:, k:k + 1],
                                          p["b0"][ts(k, 128)])
                        nc.sync.dma_start(p["b1_s"][:, k:k + 1],
                                          p["b1"][ts(k, 128)])
                    for k in range(2):
                        nc.sync.dma_start(p["w3t_s"][:, ts(k, 128)],
                                          p["w3t"][ts(k, 128), :])
                        nc.sync.dma_start(p["b2_s"][:, k:k + 1],
                                          p["b2"][ts(k, 128)])
                    nc.sync.dma_start(p["b3_s"][:, 0:1], p["b3"][:])
                for t in range(1, T):
                    prop, em = {}, {}
                    for g in pair:
                        prop[g] = p["prop_ps"].tile([65, N], F32, tag="pp",
                                                    name="pp_t")
                    for h in range(2):
                        for g in pair:
                            em[g] = _emit_iter_head(nc, p, fn_g[g], wwf_g[g],
                                                    adj_g[g], h)
                        for j in range(2):
                            jj = 2 * h + j
                            for i, g in enumerate(pair):
                                fnt_prev = fnt_tiles[fofs + 2 * i + (t - 1) % 2]
                                nc.tensor.matmul(
                                    prop[g][:],
                                    fnt_prev[:, jj * 65:(jj + 1) * 65],
                                    em[g][:, ts(j, N)], start=(jj == 0),
                                    stop=(jj == 3), skip_group_check=True)
                    for i, g in enumerate(pair):
                        fn_g[g], wwf_g[g] = _emit_iter_tail(
                            nc, p, g, t, prop[g],
                            fnt_tiles[fofs + 2 * i + t % 2], f1_g[g], gtiles)

            _emit_norm_mlp(nc, p, gtiles)

    nc.compile()
    return nc


def _prep_inputs(inputs):
    bf = ml_dtypes.bfloat16
    x = np.asarray(inputs["node_attribute_matrix"], np.float32)
    adj = np.asarray(inputs["adjacent_matrix"])
    adjf = np.ascontiguousarray((adj != 0).astype(bf))
    xT = np.ascontiguousarray(x.transpose(0, 2, 1).astype(bf))  # [B, FEAT, N]

    common = {
        "wt": np.ascontiguousarray(np.asarray(inputs["W"], np.float32).T.astype(bf)),
        "wvt": np.ascontiguousarray(np.asarray(inputs["Wv"], np.float32).T.astype(bf)),
        "wgwwt": np.ascontiguousarray(np.hstack([
            np.asarray(inputs["Wg"], np.float32).T,
            np.asarray(inputs["Ww"], np.float32).T]).astype(bf)),
        "w0t": np.ascontiguousarray(np.asarray(inputs["W0"], np.float32).T),
        "w1t": np.ascontiguousarray(np.asarray(inputs["W1"], np.float32).T),
        "w2t": np.ascontiguousarray(np.asarray(inputs["W2"], np.float32).T),
        "w3t": np.ascontiguousarray(np.asarray(inputs["W3"], np.float32).T),
        "b0": np.asarray(inputs["b0"], np.float32),
        "b1": np.asarray(inputs["b1"], np.float32),
        "b2": np.asarray(inputs["b2"], np.float32),
        "b3": np.asarray(inputs["b3"], np.float32),
        "ident": np.eye(128, dtype=np.float32),
        "identb": np.eye(128).astype(bf),
    }
    in_maps = []
    for c in range(NCORES):
        sl = slice(c * BPC, (c + 1) * BPC)
        m = dict(common)
        m["xT"] = xT[sl]
        m["adjf"] = adjf[sl]
        in_maps.append(m)
    return in_maps


def kernel(**inputs) -> np.ndarray:
    if "nc" not in _STATE:
        _STATE["nc"] = _build_program()
    nc = _STATE["nc"]
    in_maps = _prep_inputs(inputs)

    trace = bool(int(os.environ.get("GNN_TRACE", "0")))
    kwargs = {}
    if trace:
        kwargs = dict(trace=True, tmpdir=os.environ.get("GNN_TRACE_DIR") or None)
    t0 = time.time()
    res = run_bass_kernel_spmd(nc, in_maps, list(range(NCORES)), **kwargs)
    _STATE["wall_s"] = time.time() - t0
    _STATE["exec_time_ns"] = res.exec_time_ns
    _STATE["results"] = res

    out = np.empty((B, 128), np.float32)
    for c in range(NCORES):
        out[c * BPC:(c + 1) * BPC] = res.results[c]["outT"].T
    return out


# revision 35
# speedup vs baseline: 1.8819x; 1.0141x over previous
"""GNN message-passing kernel for 8 Trainium2 NeuronCores.

Full (unsharded) inputs in, full output out. Data-parallel over the batch
dimension: 64 graphs -> 8 cores x 8 graphs. Parameters replicated.

Math per graph (reference semantics):
  X  = W @ x^T                          [EMB=128, N=512]
  F1 = sigmoid(Wv @ X)                  [RP=64, N=512]
  Fn = F1
  repeat 7x:
    WwF = Ww @ Fn                       [64, 512]
    S   = Fn^T(r-contract) WwF          [512, 512]   S[n,m] = sum_r Fn[r,n] WwF[r,m]
    A   = softmax_n(mask ? -inf : S)
    Fn  = (Fn @ A) * F1
  gates q=0..7: g_q = sum_n sigmoid(Wg @ Fn_q)       [64]
  fT = concat(gates); fT /= ||fT||; out = MLP(fT)    [128]

Device implementation notes:
  * loop tensors are bf16 (validated: output error stays ~1e-4); PSUM
    accumulation is fp32 throughout.
  * sigmoid(x) = 0.5 + 0.5*tanh(x/2)  -> tanh lives in the same ACT table set
    as exp, so no table switches in the main loop.
  * softmax without max-subtraction (S is bounded); multiplicative 0/1
    adjacency mask applied to exp(S) in one fused [128, 2048] DVE op (2x
    bf16 mode).  The four S chunks live in one 4-bank PSUM tile so exp is a
    single [128, 2048] ACT op.
  * column sums Z come from an appended ones-column in the propagation
    matmul lhsT; 1/Z via fast DVE reciprocal (input staged to SBUF - the
    custom op misreads PSUM re-reads); broadcast via gpsimd.
  * gate sums come from ACT accum_out, directly assembling the transposed
    feature matrix fTT [512, 8] used by the MLP.
  * rsqrt for the norm via bit-trick seed + Newton (no sqrt table needed).
"""

import os
import time

import ml_dtypes
import numpy as np

import concourse.bacc as bacc
import concourse.tile as tile
import concourse.mybir as mybir
from concourse.bass_utils import run_bass_kernel_spmd

F32 = mybir.dt.float32
BF16 = mybir.dt.bfloat16
U32 = mybir.dt.uint32
AF = mybir.ActivationFunctionType
ALU = mybir.AluOpType

B, N, FEAT, EMB, RP = 64, 512, 256, 128, 64
T = 8          # MAX_WALK_LEN (1 initial gate + 7 propagation steps)
NCORES = 8
BPC = B // NCORES   # graphs per core
NCH = N // 128      # n-chunks of 128
D0 = RP * T         # 512, MLP width

_STATE = {}


def ts(i, size):
    return slice(i * size, (i + 1) * size)


def _emit_phase_a(nc, p, g, gtiles, fnt_tiles):
    """X = W @ x^T, F1 = sigmoid(Wv X), gate 0, FnT(F1). Returns F1 tile."""
    xg = p["xg"].tile([128, 2 * N], BF16)
    for k in range(2):
        nc.sync.dma_start(xg[:, ts(k, N)], p["xT"][g, ts(k, 128), :])

    x_ps = p["s_ps"].tile([128, 2 * N], F32, tag="s", name="s_t")[:, 0:N]
    for k in range(2):
        nc.tensor.matmul(x_ps[:], p["wt_s"][:, ts(k, 128)], xg[:, ts(k, N)],
                         start=(k == 0), stop=(k == 1))
    x_s = p["xs"].tile([128, N], BF16)
    nc.scalar.copy(x_s[:], x_ps[:])

    f1_ps = p["s_ps"].tile([128, 2 * N], F32, tag="s", name="s_t")[0:64, 0:N]
    nc.tensor.matmul(f1_ps[:], p["wvt_s"][:], x_s[:], start=True, stop=True)
    scr = p["scr"].tile([64, N], F32)
    nc.scalar.activation(scr[:], f1_ps[:], AF.Tanh, scale=0.5)
    f1 = p["f1"].tile([64, N], BF16)
    nc.vector.tensor_scalar(f1[:], scr[:], 0.5, 0.5, ALU.mult, ALU.add)

    wwf_s = _emit_combined(nc, p, g, 0, f1, gtiles)
    _emit_fnt(nc, p, f1, fnt_tiles[0])
    return f1, wwf_s


def _emit_combined(nc, p, g, q, fn, gtiles):
    """One matmul computes both the gate pre-activation (rows 0:64, lhsT WgT)
    and the next iteration's WwF (rows 64:128, lhsT WwT).  gate_q =
    sum_n sigmoid(Wg Fn) = 256 + 0.5*sum_n tanh(0.5*(Wg Fn)); the raw
    tanh-sum accumulates into the transposed-feature assembly tile and the
    affine (0.5, +256) is applied in the normalization phase.
    Returns WwF in SBUF (bf16)."""
    cmm = p["prop_ps"].tile([128, N], F32, tag="pp", name="pp_t")
    nc.tensor.matmul(cmm[:], p["wgwwt_s"][:], fn[:], start=True, stop=True)
    scr = p["scr"].tile([64, N], F32)
    half = (q % 2) * 64
    acc = gtiles[q // 2][half:half + 64, g:g + 1]
    nc.scalar.activation(scr[:], cmm[0:64, :], AF.Tanh, scale=0.5,
                         accum_out=acc)
    wwf_s = p["wwf"].tile([64, N], BF16)
    nc.vector.tensor_copy(wwf_s[:], cmm[64:128, :])
    return wwf_s


def _emit_fnt(nc, p, fn, fnt):
    """PE-transpose Fn [64,512] into fnt chunks [128,64] (cols j*65..j*65+63).
    Column j*65+64 holds the persistent ones used for the Z row."""
    fnt_ps = p["fnt_ps"].tile([128, 4 * 64], BF16, tag="pp", name="pp_t")
    for j in range(NCH):
        nc.tensor.transpose(fnt_ps[:, ts(j, 64)], fn[:, ts(j, 128)],
                            p["identb_s"][0:64, 0:64])
    # one strided copy: chunks land at stride-65 offsets, skipping ones cols
    dst = fnt.rearrange("p (c k) -> p c k", k=65)[:, :, 0:64]
    src = fnt_ps[:].rearrange("p (c k) -> p c k", k=64)
    nc.vector.tensor_copy(dst, src)


def _emit_iter_head(nc, p, fn_prev, wwf_s, adj_g, h):
    """S-chunk half h -> exp -> mask. Returns masked exp tile [128, 1024]."""
    s_h = p["s_ps"].tile([128, 2 * N], F32, tag="s", name="s_t")
    for j in range(2):
        nc.tensor.matmul(s_h[:, ts(j, N)],
                         fn_prev[:, ts(2 * h + j, 128)], wwf_s[:],
                         start=True, stop=True)
    e_h = p["e"].tile([128, 2 * N], BF16)
    nc.scalar.activation(e_h[:], s_h[:], AF.Exp)
    em_h = p["em"].tile([128, 2 * N], BF16)
    nc.vector.tensor_tensor(em_h[:], e_h[:],
                            adj_g[:, ts(h, 2 * N)], ALU.mult)
    return em_h


def _emit_iter_tail(nc, p, g, t, prop, fnt_next, f1, gtiles):
    """Softmax normalization + Fn update + gate/WwF + FnT transposes."""
    zsb = p["zsb"].tile([1, N], F32)
    nc.scalar.copy(zsb[:], prop[64:65, :])
    rz = p["rz"].tile([1, N], F32)
    nc.vector.reciprocal_approx_fast(rz[:], zsb[:])
    zb = p["zb"].tile([64, N], F32)
    nc.gpsimd.partition_broadcast(zb[:], rz[:])
    fnuf1 = p["fnuf1"].tile([64, N], F32)
    nc.vector.tensor_tensor(fnuf1[:], prop[0:64, :], f1[:], ALU.mult)
    fn_new = p["fn"].tile([64, N], BF16)
    nc.vector.tensor_tensor(fn_new[:], fnuf1[:], zb[:], ALU.mult)

    wwf_new = _emit_combined(nc, p, g, t, fn_new, gtiles)
    if t < T - 1:
        _emit_fnt(nc, p, fn_new, fnt_next)
    return fn_new, wwf_new


def _emit_norm_mlp(nc, p, gtiles):
    """Gate affine + L2 normalization + 4-layer MLP, all graphs at once."""
    ident = p["ident_s"]
    # transpose the 4 assembly tiles [128, BPC] -> row layout [BPC, 512]
    row_ps = p["sm_ps"].tile([BPC, N], F32, tag="pp", name="pp_t")
    for j in range(NCH):
        nc.tensor.transpose(row_ps[:, ts(j, 128)], gtiles[j][:, 0:BPC],
                            ident[:, 0:128])
    f_row = p["frow"].tile([BPC, N], F32)
    # gate = 0.5*acc + 256 applied during the PSUM->SBUF copy
    nc.vector.tensor_scalar(f_row[:], row_ps[:], 0.5, 256.0, ALU.mult, ALU.add)

    # ss[g] = sum_k f_row[g,k]^2 via Square activation with accumulate
    sq = p["frow"].tile([BPC, N], F32)
    ss = p["tiny"].tile([BPC, 1], F32, tag="ss")
    nc.scalar.activation(sq[:], f_row[:], AF.Square, accum_out=ss[:])

    # rn = rsqrt(ss): bit-trick sqrt seed, fast reciprocal, 3 Newton steps
    tmp = p["tiny"].tile([BPC, 1], F32, tag="t0")
    y = p["tiny"].tile([BPC, 1], F32, tag="t1")
    a = p["tiny"].tile([BPC, 1], F32, tag="t2")
    nc.vector.tensor_scalar(tmp[:].bitcast(U32), ss[:].bitcast(U32),
                            1, None, ALU.logical_shift_right)
    nc.vector.tensor_scalar(tmp[:].bitcast(U32), tmp[:].bitcast(U32),
                            0x1FBD1DF5, None, ALU.add)
    nc.vector.reciprocal_approx_fast(y[:], tmp[:])
    for _ in range(3):
        nc.vector.tensor_tensor(a[:], y[:], y[:], ALU.mult)       # y^2
        nc.vector.tensor_tensor(a[:], ss[:], a[:], ALU.mult)      # ss*y^2
        nc.vector.tensor_scalar(a[:], a[:], -0.5, 1.5, ALU.mult, ALU.add)
        nc.vector.tensor_tensor(y[:], y[:], a[:], ALU.mult)

    fn_row = p["frow"].tile([BPC, N], F32)
    nc.vector.tensor_scalar(fn_row[:], f_row[:], y[:], None, ALU.mult)

    # back to transposed layout [512, BPC] for the MLP
    h0 = p["mlp"].tile([128, 4 * BPC], F32, tag="h0")
    for j in range(NCH):
        t_ps = p["sm_ps"].tile([128, BPC], F32, tag="pp", name="pp_t")
        nc.tensor.transpose(t_ps[:, 0:BPC], fn_row[:, ts(j, 128)],
                            ident[0:BPC, 0:BPC])
        nc.vector.tensor_copy(h0[:, ts(j, BPC)], t_ps[:, 0:BPC])

    # MLP in transposed layout: h_next[j,g] = act(sum_k WT[k,j] h[k,g] + b[j])
    def layer(h_in, kch, jch, w_s, b_s, act, tag):
        h_out = p["mlp"].tile([128, jch * BPC], F32, tag=tag)
        for j in range(jch):
            mm = p["sm_ps"].tile([128, BPC], F32, tag="pp", name="pp_t")
            for k in range(kch):
                nc.tensor.matmul(mm[:], w_s[:, k * (jch * 128) + j * 128:
                                             k * (jch * 128) + (j + 1) * 128],
                                 h_in[:, ts(k, BPC)],
                                 start=(k == 0), stop=(k == kch - 1))
            nc.scalar.activation(h_out[:, ts(j, BPC)], mm[:], act,
                                 bias=b_s[:, j:j + 1])
        return h_out

    h1 = layer(h0, 4, 4, p["w0t_s"], p["b0_s"], AF.Relu, "h1")
    h2 = layer(h1, 4, 4, p["w1t_s"], p["b1_s"], AF.Relu, "h2")
    h3 = layer(h2, 4, 2, p["w2t_s"], p["b2_s"], AF.Relu, "h3")
    h4 = layer(h3, 2, 1, p["w3t_s"], p["b3_s"], AF.Identity, "h4")
    nc.sync.dma_start(p["outT"][:, :], h4[:, 0:BPC])


def _build_program():
    nc = bacc.Bacc("TRN2", target_bir_lowering=False, debug=False,
                   num_devices=NCORES)
    p = {}
    p["xT"] = nc.dram_tensor("xT", [BPC, FEAT, N], BF16, kind="ExternalInput").ap()
    p["adjf"] = nc.dram_tensor("adjf", [BPC, N, N], BF16, kind="ExternalInput").ap()
    for name, shape, dt in [("wt", [FEAT, EMB], BF16), ("wvt", [EMB, RP], BF16),
                            ("wgwwt", [RP, 2 * RP], BF16),
                            ("w0t", [D0, D0], F32), ("w1t", [D0, D0], F32),
                            ("w2t", [D0, D0 // 2], F32),
                            ("w3t", [D0 // 2, 128], F32),
                            ("b0", [D0], F32), ("b1", [D0], F32),
                            ("b2", [D0 // 2], F32), ("b3", [128], F32),
                            ("ident", [128, 128], F32),
                            ("identb", [128, 128], BF16)]:
        p[name] = nc.dram_tensor(name, shape, dt, kind="ExternalInput").ap()
    p["outT"] = nc.dram_tensor("outT", [128, BPC], F32, kind="ExternalOutput").ap()

    with tile.TileContext(nc) as tc:
        import contextlib
        with contextlib.ExitStack() as ctx:
            const = ctx.enter_context(tc.tile_pool(name="const", bufs=1))
            p["wt_s"] = const.tile([128, 2 * 128], BF16, tag="wt", name="wt_s")
            p["wvt_s"] = const.tile([128, 64], BF16, tag="wvt", name="wvt_s")
            p["wgwwt_s"] = const.tile([64, 128], BF16, tag="wgwwt",
                                      name="wgwwt_s")
            p["ident_s"] = const.tile([128, 128], F32, tag="ident", name="ident_s")
            p["identb_s"] = const.tile([128, 128], BF16, tag="identb",
                                       name="identb_s")
            p["w0t_s"] = const.tile([128, 4 * D0], F32, tag="w0t", name="w0t_s")
            p["w1t_s"] = const.tile([128, 4 * D0], F32, tag="w1t", name="w1t_s")
            p["w2t_s"] = const.tile([128, 4 * (D0 // 2)], F32, tag="w2t",
                                    name="w2t_s")
            p["w3t_s"] = const.tile([128, 2 * 128], F32, tag="w3t", name="w3t_s")
            p["b0_s"] = const.tile([128, 4], F32, tag="b0", name="b0_s")
            p["b1_s"] = const.tile([128, 4], F32, tag="b1", name="b1_s")
            p["b2_s"] = const.tile([128, 2], F32, tag="b2", name="b2_s")
            p["b3_s"] = const.tile([128, 1], F32, tag="b3", name="b3_s")
            gtiles = [const.tile([128, BPC], F32, tag=f"gt{i}", name=f"gt{i}")
                      for i in range(4)]
            fnt_tiles = [const.tile([128, 4 * 65], BF16, tag=f"fnt{i}",
                                    name=f"fnt{i}")
                         for i in range(8)]  # 2 ping-pong x 2 graphs x 2 pairs

            for k in range(2):
                nc.sync.dma_start(p["wt_s"][:, ts(k, 128)], p["wt"][ts(k, 128), :])
            nc.sync.dma_start(p["wvt_s"][:], p["wvt"][:, :])
            nc.sync.dma_start(p["wgwwt_s"][:], p["wgwwt"][:, :])
            nc.sync.dma_start(p["ident_s"][:], p["ident"][:, :])
            nc.sync.dma_start(p["identb_s"][:], p["identb"][:, :])
            for i in range(4):
                nc.gpsimd.memset(gtiles[i][:], 0.0)
            for i in range(8):
                for j in range(NCH):
                    nc.gpsimd.memset(
                        fnt_tiles[i][:, j * 65 + 64:j * 65 + 65], 1.0)

            p["xg"] = ctx.enter_context(tc.tile_pool(name="xg", bufs=2))
            p["adj"] = ctx.enter_context(tc.tile_pool(name="adj", bufs=3))
            p["xs"] = ctx.enter_context(tc.tile_pool(name="xs", bufs=2))
            p["f1"] = ctx.enter_context(tc.tile_pool(name="f1", bufs=2))
            p["fn"] = ctx.enter_context(tc.tile_pool(name="fn", bufs=4))
            p["scr"] = ctx.enter_context(tc.tile_pool(name="scr", bufs=3))
            p["wwf"] = ctx.enter_context(tc.tile_pool(name="wwf", bufs=3))
            p["e"] = ctx.enter_context(tc.tile_pool(name="e", bufs=3))
            p["em"] = ctx.enter_context(tc.tile_pool(name="em", bufs=3))
            p["rz"] = ctx.enter_context(tc.tile_pool(name="rz", bufs=2))
            p["zsb"] = ctx.enter_context(tc.tile_pool(name="zsb", bufs=2))
            p["zb"] = ctx.enter_context(tc.tile_pool(name="zb", bufs=2))
            p["fnuf1"] = ctx.enter_context(tc.tile_pool(name="fnuf1", bufs=2))
            p["frow"] = ctx.enter_context(tc.tile_pool(name="frow", bufs=1))
            p["tiny"] = ctx.enter_context(tc.tile_pool(name="tiny", bufs=1))
            p["mlp"] = ctx.enter_context(tc.tile_pool(name="mlp", bufs=1))
            # PSUM: 2x2 (S halves) + 3 (prop/combined/fnt) + 1 (misc) = 8
            p["s_ps"] = ctx.enter_context(
                tc.tile_pool(name="s_ps", bufs=2, space="PSUM"))
            p["prop_ps"] = ctx.enter_context(
                tc.tile_pool(name="prop_ps", bufs=4, space="PSUM"))
            p["sm_ps"] = p["prop_ps"]
            p["fnt_ps"] = p["prop_ps"]

            for gp in range(BPC // 2):
                pair = (2 * gp, 2 * gp + 1)
                fofs = 4 * (gp % 2)
                adj_g, f1_g, fn_g = {}, {}, {}
                for g in pair:
                    ag = p["adj"].tile([128, NCH * N], BF16)
                    for j in range(NCH):
                        nc.sync.dma_start(ag[:, ts(j, N)],
                                          p["adjf"][g, ts(j, 128), :])
                    adj_g[g] = ag
                wwf_g = {}
                for i, g in enumerate(pair):
                    f1_g[g], wwf_g[g] = _emit_phase_a(
                        nc, p, g, gtiles,
                        fnt_tiles[fofs + 2 * i:fofs + 2 * i + 2])
                    fn_g[g] = f1_g[g]
                if gp == 0:
                    # MLP weights are only needed at the very end; loading
                    # them here keeps the startup DMAs for pair 0 unblocked
                    for k in range(4):
                        nc.sync.dma_start(p["w0t_s"][:, ts(k, D0)],
                                          p["w0t"][ts(k, 128), :])
                        nc.sync.dma_start(p["w1t_s"][:, ts(k, D0)],
                                          p["w1t"][ts(k, 128), :])
                        nc.sync.dma_start(p["w2t_s"][:, ts(k, D0 // 2)],
                                          p["w2t"][ts(k, 128), :])
                        nc.sync.dma_start(p["b0_s"][:, k:k + 1],
                                          p["b0"][ts(k, 128)])
                        nc.sync.dma_start(p["b1_s"][:, k:k + 1],
                                          p["b1"][ts(k, 128)])
                    for k in range(2):
                        nc.sync.dma_start(p["w3t_s"][:, ts(k, 128)],
                                          p["w3t"][ts(k, 128), :])
                        nc.sync.dma_start(p["b2_s"][:, k:k + 1],
                                          p["b2"][ts(k, 128)])
                    nc.sync.dma_start(p["b3_s"][:, 0:1], p["b3"][:])
                for t in range(1, T):
                    prop, em = {}, {}
                    for g in pair:
                        prop[g] = p["prop_ps"].tile([65, N], F32, tag="pp",
                                                    name="pp_t")
                    for h in range(2):
                        for g in pair:
                            em[g] = _emit_iter_head(nc, p, fn_g[g], wwf_g[g],
                                                    adj_g[g], h)
                        for j in range(2):
                            jj = 2 * h + j
                            for i, g in enumerate(pair):
                                fnt_prev = fnt_tiles[fofs + 2 * i + (t - 1) % 2]
                                nc.tensor.matmul(
                                    prop[g][:],
                                    fnt_prev[:, jj * 65:(jj + 1) * 65],
                                    em[g][:, ts(j, N)], start=(jj == 0),
                                    stop=(jj == 3), skip_group_check=True)
                    for i, g in enumerate(pair):
                        fn_g[g], wwf_g[g] = _emit_iter_tail(
                            nc, p, g, t, prop[g],
                            fnt_tiles[fofs + 2 * i + t % 2], f1_g[g], gtiles)

            _emit_norm_mlp(nc, p, gtiles)

    nc.compile()
    return nc


def _prep_inputs(inputs):
    bf = ml_dtypes.bfloat16
    x = np.asarray(inputs["node_attribute_matrix"], np.float32)
    adj = np.asarray(inputs["adjacent_matrix"])
    adjf = np.ascontiguousarray((adj != 0).astype(bf))
    xT = np.ascontiguousarray(x.transpose(0, 2, 1).astype(bf))  # [B, FEAT, N]

    common = {
        "wt": np.ascontiguousarray(np.asarray(inputs["W"], np.float32).T.astype(bf)),
        "wvt": np.ascontiguousarray(np.asarray(inputs["Wv"], np.float32).T.astype(bf)),
        "wgwwt": np.ascontiguousarray(np.hstack([
            np.asarray(inputs["Wg"], np.float32).T,
            np.asarray(inputs["Ww"], np.float32).T]).astype(bf)),
        "w0t": np.ascontiguousarray(np.asarray(inputs["W0"], np.float32).T),
        "w1t": np.ascontiguousarray(np.asarray(inputs["W1"], np.float32).T),
        "w2t": np.ascontiguousarray(np.asarray(inputs["W2"], np.float32).T),
        "w3t": np.ascontiguousarray(np.asarray(inputs["W3"], np.float32).T),
        "b0": np.asarray(inputs["b0"], np.float32),
        "b1": np.asarray(inputs["b1"], np.float32),
        "b2": np.asarray(inputs["b2"], np.float32),
        "b3": np.asarray(inputs["b3"], np.float32),
        "ident": np.eye(128, dtype=np.float32),
        "identb": np.eye(128).astype(bf),
    }
    in_maps = []
    for c in range(NCORES):
        sl = slice(c * BPC, (c + 1) * BPC)
        m = dict(common)
        m["xT"] = xT[sl]
        m["adjf"] = adjf[sl]
        in_maps.append(m)
    return in_maps


def kernel(**inputs) -> np.ndarray:
    if "nc" not in _STATE:
        _STATE["nc"] = _build_program()
    nc = _STATE["nc"]
    in_maps = _prep_inputs(inputs)

    trace = bool(int(os.environ.get("GNN_TRACE", "0")))
    kwargs = {}
    if trace:
        kwargs = dict(trace=True, tmpdir=os.environ.get("GNN_TRACE_DIR") or None)
    t0 = time.time()
    res = run_bass_kernel_spmd(nc, in_maps, list(range(NCORES)), **kwargs)
    _STATE["wall_s"] = time.time() - t0
    _STATE["exec_time_ns"] = res.exec_time_ns
    _STATE["results"] = res

    out = np.empty((B, 128), np.float32)
    for c in range(NCORES):
        out[c * BPC:(c + 1) * BPC] = res.results[c]["outT"].T
    return out
